# revision 36
# baseline (speedup 1.0000x reference)
"""Trainium2 Bass kernel: causal multi-head attention with RoPE.

Model: B=4, L=2048, H=2048, NH=16 heads, head_dim=128.
  q = x @ Wq.T ; k = x @ Wk.T ; v = x @ Wv.T        (per-head split)
  q, k <- RoPE(q, k)
  attn = softmax(mask(q k^T / sqrt(hd)))
  out  = (attn @ v) heads-concat @ Wo.T

Sharding (8 cores): hybrid batch x tensor-parallel.  Core c handles
batch b = c//2 and heads half*8..half*8+7 with half = c%2.  Wq/Wk/Wv are
column-sharded (8 heads per core), Wo row-sharded; each core produces a
partial y[b] and the host sums the two partials per batch (the unshard
step) and concatenates batches.

Per-core dataflow (all SBUF-resident, bf16 inputs / fp32 accumulation):
  phase A: Q^T, K^T  [128d x 2048pos] per head (d-major) and V
           [128pos x 1024d] pos-major, via PE matmuls; RoPE on Q^T/K^T
           (rotate-half partition shuffle via SBUF->SBUF DMA, the
           elementwise part on DVE).
  phase B: flash-style causal attention per (head, 512-wide q chunk):
           S^T tile = K_blk^T Q_chunk (PE), P = exp(S^T/sqrt(d)) (ACT),
           block-sparse causal structure with a triangular-mask multiply
           on diagonal blocks (DVE), O^T += V_blk P (PE), rowsum via
           ones-matmul (PE), reciprocal+broadcast+scale for the softmax
           normalization (DVE + GPSIMD).
  phase C: y^T partial = Wo_shard O^T (PE) -> DRAM fp32.
"""

import math
import numpy as np

B, L, H, NH, HD = 4, 2048, 2048, 16, 128
ROPE_BASE = 10000.0
NCORES = 8
HPC = 8          # heads per core
QC = 512         # q chunk width
NQC = L // QC    # 4 q chunks
NKB = L // 128   # 16 kp blocks
SCALE = 1.0 / math.sqrt(HD)

_cache = {}


def _analyze_mask(mask2d):
    """Classify each (q_block, kp_block) 128x128 block of the [L, L] mask.

    Returns (block_kind[16][16] with 0=empty,1=full,2=mixed, patterns,
    pattern_idx dict keyed by block coords). mask2d is int32 [L, L],
    rows=q, cols=kp.
    """
    nb = L // 128
    kind = [[0] * nb for _ in range(nb)]
    patterns = []
    pat_key_to_idx = {}
    block_pat = {}
    for qb in range(nb):
        rows = mask2d[qb * 128:(qb + 1) * 128]
        for kb in range(nb):
            blk = rows[:, kb * 128:(kb + 1) * 128]
            s = int(blk.sum())
            if s == 0:
                kind[qb][kb] = 0
            elif s == 128 * 128:
                kind[qb][kb] = 1
            else:
                kind[qb][kb] = 2
                key = blk.tobytes()
                idx = pat_key_to_idx.get(key)
                if idx is None:
                    idx = len(patterns)
                    pat_key_to_idx[key] = idx
                    # stored transposed: S^T tiles are [kp, q]
                    patterns.append(np.ascontiguousarray(blk.T))
                block_pat[(qb, kb)] = idx
    return kind, patterns, block_pat


def _build(kind, block_pat, n_patterns):
    """Build the SPMD bass program (same for all 8 cores)."""
    import concourse.bass as bass
    import concourse.bacc as bacc
    import concourse.mybir as mybir
    import concourse.tile as tile

    fp32 = mybir.dt.float32
    bf16 = mybir.dt.bfloat16
    EXP = mybir.ActivationFunctionType.Exp

    nc = bacc.Bacc("TRN2", target_bir_lowering=False, debug=False)

    NHC = H // 128  # 16 input-feature blocks

    # All big operands are pre-shuffled on the host into partition-major
    # layouts with LONG contiguous per-partition runs so DMA packets are
    # 8-32KB instead of 1-2KB (per-queue DMA throughput is packet-rate
    # limited).
    xP = nc.dram_tensor("xP", [NQC, 128, NHC * QC], bf16,
                        kind="ExternalInput")
    wqT = nc.dram_tensor("wqT", [128, NHC, HPC * HD], bf16,
                         kind="ExternalInput")
    wkT = nc.dram_tensor("wkT", [128, NHC, HPC * HD], bf16,
                         kind="ExternalInput")
    wvT = nc.dram_tensor("wvT", [128, NHC, HPC * HD], bf16,
                         kind="ExternalInput")
    woT = nc.dram_tensor("woT", [128, HPC, H], bf16, kind="ExternalInput")
    cosd = nc.dram_tensor("cosd", [HD, L], bf16, kind="ExternalInput")
    sinmd = nc.dram_tensor("sinmd", [HD, L], bf16, kind="ExternalInput")
    npat = max(n_patterns, 1)
    maskd = nc.dram_tensor("maskd", [npat, 128, 128], bf16, kind="ExternalInput")
    yT = nc.dram_tensor("yT", [H, L], bf16, kind="ExternalOutput")

    def qk_phase(tc, w_dram, out_a, wpool, xpool, tpool, pspool, wtag,
                 cos_sb, sinm_sb, delay_after=None, after_x0=None):
        """Q^T / K^T d-major projection + fused RoPE per (head, chunk).

        delay_after: instructions the weight DMA must wait for -- used to
        keep the K-phase weight prefetch off the critical head-of-kernel
        DMA bandwidth.  Weight DMAs ride the scalar HWDGE queue so they
        run concurrently with x DMAs on the sync queue.
        """
        from concourse.tile import add_dep_helper
        w_sb = wpool.tile([128, NHC, HPC * HD], bf16, tag="w",
                          name=f"w_{wtag}")
        w_insts = []
        for g in range(4):
            w_insts.append(
                nc.scalar.dma_start(out=w_sb[:, 4 * g:4 * g + 4, :],
                                    in_=w_dram[:, 4 * g:4 * g + 4, :]))
        if delay_after:
            for wi in w_insts:
                for di in delay_after:
                    add_dep_helper(wi.ins, di.ins, reason="defer weight prefetch")
        x0_insts = []
        # RoPE for head h is emitted one head late so the DVE queue (strict
        # 8-deep FIFO) never head-blocks on the rotate-half DMA latency.
        rope_q = []

        def emit_rope(h, js):
            q = out_a[:, h, js]
            rq = tpool.tile([128, QC], bf16, tag="rotq")
            eng = nc.sync if h % 2 == 0 else nc.scalar
            eng.dma_start(out=rq[0:64, :], in_=out_a[64:128, h, js])
            eng.dma_start(out=rq[64:128, :], in_=out_a[0:64, h, js])

            def fire():
                nc.vector.tensor_mul(rq[:], rq[:], sinm_sb[:, js])
                nc.vector.tensor_mul(q, q, cos_sb[:, js])
                nc.vector.tensor_add(q, q, rq[:])
            rope_q.append(fire)
            if len(rope_q) > 1:
                rope_q.pop(0)()

        for j in range(NQC):
            js = slice(j * QC, (j + 1) * QC)
            x_sb = xpool.tile([128, NHC, QC], bf16, tag="xcols",
                              name=f"x_{wtag}{j}")
            for g in range(4):
                di = nc.sync.dma_start(
                    out=x_sb[:, 4 * g:4 * g + 4, :],
                    in_=xP[j, :, 4 * g * QC:(4 * g + 4) * QC])
                if j == 0:
                    x0_insts.append(di)
            if j == 0 and after_x0 is not None:
                after_x0()
            for h in range(HPC):
                ps = pspool.tile([128, QC], fp32, tag="ps_proj")
                for hc in range(NHC):
                    nc.tensor.matmul(
                        ps[:],
                        w_sb[:, hc, h * HD:(h + 1) * HD],
                        x_sb[:, hc, :],
                        start=(hc == 0), stop=(hc == NHC - 1))
                nc.vector.tensor_copy(out_a[:, h, js], ps[:])
                emit_rope(h, js)
        while rope_q:
            rope_q.pop(0)()
        return x0_insts

    def v_phase(tc, w_dram, va, wpool, xpool, pspool, delay_after=None):
        """V pos-major projection (same pre-shuffled x chunks as Q/K)."""
        from concourse.tile import add_dep_helper
        w_sb = wpool.tile([128, NHC, HPC * HD], bf16, tag="w", name="w_v")
        for g in range(4):
            nc.scalar.dma_start(out=w_sb[:, 4 * g:4 * g + 4, :],
                                in_=w_dram[:, 4 * g:4 * g + 4, :])
        for j in range(NQC):
            x_sb = xpool.tile([128, NHC, QC], bf16, tag="xv", name=f"xv{j}")
            for g in range(4):
                di = nc.sync.dma_start(
                    out=x_sb[:, 4 * g:4 * g + 4, :],
                    in_=xP[j, :, 4 * g * QC:(4 * g + 4) * QC])
                if delay_after and j == 0:
                    for d0 in delay_after:
                        add_dep_helper(di.ins, d0.ins,
                                       reason="defer xv prefetch")
            for pb in range(QC // 128):
                psd = [pspool.tile([128, QC], fp32, tag="ps_proj",
                                   name=f"psv{j}_{pb}_{dc}")
                       for dc in range(2)]
                for hc in range(NHC):
                    for dc in range(2):
                        nc.tensor.matmul(
                            psd[dc][:],
                            x_sb[:, hc, pb * 128:(pb + 1) * 128],
                            w_sb[:, hc, dc * QC:(dc + 1) * QC],
                            start=(hc == 0), stop=(hc == NHC - 1))
                for dc in range(2):
                    nc.vector.tensor_copy(
                        va[:, j * (QC // 128) + pb, dc * QC:(dc + 1) * QC],
                        psd[dc][:])

    with tile.TileContext(nc) as tc:
        with tc.tile_pool(name="persist", bufs=1, side="left") as persist:
            # one combined small-constant tile: [trimask patterns | ones]
            cst = persist.tile([128, npat * 128 + 128], bf16, tag="cst")
            ones_col = npat * 128
            nc.vector.memset(cst[:, ones_col:ones_col + 128], 1.0)
            onesf = persist.tile([128, 128], fp32, tag="onesf")
            nc.vector.memset(onesf[:], 1.0)
            # warm the gpsimd partition-ops library now (PE is idle waiting
            # on DMAs anyway); the first real partition_broadcast otherwise
            # stalls attention ~9us on LOAD_LIB.
            nc.gpsimd.partition_broadcast(onesf[:, 0:8], onesf[0:1, 0:8])
            QTa = persist.tile([HD, HPC, L], bf16, tag="qta")
            KTa = persist.tile([HD, HPC, L], bf16, tag="kta")

            # ---------------- phase A: projections + RoPE ----------------
            # Manual pool lifetimes (non-LIFO): weights/x/rope tables are
            # freed before attention while Va spans V-phase..attention.
            wpool_cm = tc.tile_pool(name="wpool", bufs=2, side="right")
            wpool = wpool_cm.__enter__()
            ropec_cm = tc.tile_pool(name="ropec", bufs=1, side="right")
            ropec = ropec_cm.__enter__()
            psp_cm = tc.tile_pool(name="ps_proj", bufs=4, space="PSUM")
            psp = psp_cm.__enter__()

            cos_sb = ropec.tile([HD, L], bf16, tag="cos")
            sinm_sb = ropec.tile([HD, L], bf16, tag="sinm")

            def rope_dma():
                # emitted after the first x chunk so the critical-path
                # startup DMAs (x g0 + wq g0) own the HBM bandwidth
                nc.sync.dma_start(out=cos_sb[:], in_=cosd[:])
                nc.sync.dma_start(out=sinm_sb[:], in_=sinmd[:])

            xv_cm = tc.tile_pool(name="xv", bufs=2, side="right")
            xv = xv_cm.__enter__()
            xqk_cm = tc.tile_pool(name="xqk", bufs=2, side="right")
            xqk = xqk_cm.__enter__()
            tpool_cm = tc.tile_pool(name="tpool", bufs=2, side="right")
            tpool = tpool_cm.__enter__()
            q_x0 = qk_phase(tc, wqT, QTa, wpool, xqk, tpool, psp, "q",
                            cos_sb, sinm_sb, after_x0=rope_dma)
            qk_phase(tc, wkT, KTa, wpool, xqk, tpool, psp, "k",
                     cos_sb, sinm_sb, delay_after=q_x0)
            # tri-mask patterns: needed only in attention; scalar queue
            # behind the wq/wk weight loads
            for p in range(n_patterns):
                nc.scalar.dma_start(out=cst[:, p * 128:(p + 1) * 128],
                                    in_=maskd[p])
            tpool_cm.__exit__(None, None, None)
            xqk_cm.__exit__(None, None, None)
            vp_cm = tc.tile_pool(name="vp", bufs=1, side="left")
            vp_outer = vp_cm.__enter__()
            Va = vp_outer.tile([128, NKB, HPC * HD], bf16, tag="va")
            v_phase(tc, wvT, Va, wpool, xv, psp, delay_after=q_x0)
            xv_cm.__exit__(None, None, None)
            ropec_cm.__exit__(None, None, None)
            wpool_cm.__exit__(None, None, None)
            psp_cm.__exit__(None, None, None)

            # -------- phase B + C under Va's lifetime --------
            _attn_and_out(tc, nc, kind, block_pat, QTa, KTa, Va,
                          cst, ones_col, onesf, woT, yT, fp32, bf16, EXP)
            vp_cm.__exit__(None, None, None)

    nc.compile()
    return nc


def _attn_and_out(tc, nc, kind, block_pat, QTa, KTa, Va, cst, ones_col,
                  onesf, woT, yT, fp32, bf16, EXP):
    with tc.tile_pool(name="otp", bufs=1, side="left") as otp, \
         tc.tile_pool(name="wo", bufs=1, side="left") as wop:
        OTa = otp.tile([HD, HPC, L], bf16, tag="ota")
        wo_sb = wop.tile([128, HPC, H], bf16, tag="wo")
        # prefetch Wo during attention, split across both HWDGE queues
        nc.scalar.dma_start(out=wo_sb[:, 0:4, :], in_=woT[:, 0:4, :])
        nc.sync.dma_start(out=wo_sb[:, 4:8, :], in_=woT[:, 4:8, :])

        # ---------------- phase B: attention ----------------
        # q-chunk PAIRS inside the kp-block loop.  Per (i, jpair) both S
        # tiles land in one 2-bank PSUM tile so a single ACT exp covers
        # them (ACT cost is (N+~310)/1.2 ns -- instruction count matters);
        # the O accumulators and the running softmax-denominator Pacc are
        # pair-wide too.  The denominator is finished by a gpsimd
        # partition_all_reduce (cross-partition sum, idle engine) followed
        # by a DVE reciprocal, and the normalization multiplies PSUM O by
        # the all-reduced reciprocal directly -- no ones-matmul, no
        # partition_broadcast, no PSUM rowsum banks.  The last O/normalize
        # group of each pair is deferred into the next pair's instruction
        # stream (software pipeline) so the PE never head-of-line blocks.
        import concourse.bass_isa as bass_isa
        with tc.tile_pool(name="pp", bufs=4, side="right") as ppool, \
             tc.tile_pool(name="aa", bufs=2, side="right") as apool, \
             tc.tile_pool(name="rs", bufs=2, side="right") as rspool, \
             tc.tile_pool(name="ysb", bufs=4, side="right") as ypool, \
             tc.tile_pool(name="ps_s", bufs=2, space="PSUM") as ps_s, \
             tc.tile_pool(name="ps_o", bufs=2, space="PSUM") as ps_o:
            pending = []   # deferred last-O emissions (flushed next iter)
            norm_q = []    # deferred recip+normalize closures, with age

            # ---- output-projection units (phase C, interleavable) ----
            # one unit = (q-chunk j, pair of output blocks): 16 matmuls
            # into a 2-bank PSUM tile from the shared pss ring, one ACT
            # copy to SBUF, one fat DMA out.  Chunk 0/1 units interleave
            # into the (2,3)-pair attention stream; chunk 2/3 units run
            # after attention.
            def op_unit(j, op):
                def emit():
                    ps = ps_s.tile([128, 2 * QC], fp32, tag="pss",
                                   name=f"psc{j}_{op}")
                    for koc in range(2):
                        oc = 2 * op + koc
                        for fc in range(HPC):
                            nc.tensor.matmul(
                                ps[:, koc * QC:(koc + 1) * QC],
                                wo_sb[:, fc, oc * 128:(oc + 1) * 128],
                                OTa[:, fc, j * QC:(j + 1) * QC],
                                start=(fc == 0), stop=(fc == HPC - 1))
                    y_sb = ypool.tile([128, 2 * QC], bf16, tag="y")
                    nc.scalar.copy(y_sb[:], ps[:])
                    yr = yT[2 * op * 128:(2 * op + 2) * 128,
                            j * QC:(j + 1) * QC].rearrange(
                                "(a p) m -> p a m", p=128)
                    eng = nc.sync if op % 2 == 0 else nc.scalar
                    eng.dma_start(out=yr, in_=y_sb[:])
                return emit

            op_units = [op_unit(j, op) for j in (0, 1)
                        for op in range(H // 256)]

            def emit_ovr(ctx, i, group):
                h = ctx["h"]
                # O matmuls first (V stationary shared across the pair)
                for j, jj, P, w0, first in group:
                    m0 = 0 if first else w0
                    nc.tensor.matmul(
                        ctx["pso"][:, jj * QC + m0:(jj + 1) * QC],
                        Va[:, i, h * HD:(h + 1) * HD],
                        P[:, jj * QC + m0:(jj + 1) * QC],
                        start=first, stop=(ctx["last_i"][j] == i))
                if i == ctx["pair_last"]:
                    # evacuate O to SBUF on ACT right away: frees the pso
                    # PSUM tile so the next pairs never wait on the
                    # normalization chain
                    oev = rspool.tile([128, 2 * QC], bf16, tag="oev",
                                      name=f"oev{h}_{ctx['jp0']}")
                    nc.scalar.copy(oev[:], ctx["pso"][:])
                    # denominators: cross-partition sum of Pacc on gpsimd
                    # now; the DVE reciprocal+scale is deferred a few more
                    # iterations so the DVE FIFO never head-blocks on the
                    # gpsimd latency
                    rs = rspool.tile([128, 2 * QC], fp32, tag="rs",
                                     name=f"rs{h}_{ctx['jp0']}")
                    nc.gpsimd.partition_all_reduce(
                        rs[:], ctx["pacc"][:], channels=128,
                        reduce_op=bass_isa.ReduceOp.add)

                    def norm(ctx=ctx, rs=rs, oev=oev, h=h):
                        rinv = rspool.tile([128, 2 * QC], fp32, tag="rinv",
                                           name=f"rinv{h}_{ctx['jp0']}")
                        nc.vector.reciprocal_approx_fast(out=rinv[:],
                                                         in_=rs[:])
                        nc.vector.tensor_mul(
                            OTa[:, h,
                                ctx["jp0"] * QC:(ctx["jp0"] + 2) * QC],
                            oev[:], rinv[:])
                    norm_q.append([0, norm])

            def tick_norms(final=False):
                for e in norm_q:
                    e[0] += 1
                while norm_q and (final or norm_q[0][0] >= 3):
                    norm_q.pop(0)[1]()

            def flush_pending(final=False):
                while pending:
                    emit_ovr(*pending.pop(0))
                if final:
                    tick_norms(final=True)

            it23 = 0
            for jpair, h in [((0, 1), hh) for hh in range(HPC)] + \
                            [((2, 3), hh) for hh in range(HPC)]:
                if True:
                    blocks_j = {}
                    first_i = {}
                    last_i = {}
                    for j in jpair:
                        for i in range(NKB):
                            live = [t for t in range(4)
                                    if kind[4 * j + t][i] != 0]
                            if live:
                                blocks_j.setdefault(i, []).append((j, live))
                                if j not in first_i:
                                    first_i[j] = i
                                last_i[j] = i
                    if not first_i:
                        continue
                    pair_first = min(first_i.values())
                    ctx = {
                        "h": h,
                        "jp0": jpair[0],
                        "first_i": first_i,
                        "last_i": last_i,
                        "pair_last": max(last_i.values()),
                        "pso": ps_o.tile([128, 2 * QC], fp32, tag="pso",
                                         name=f"pso{h}_{jpair[0]}"),
                        "pacc": apool.tile([128, 2 * QC], bf16, tag="pacc",
                                           name=f"pacc{h}_{jpair[0]}"),
                    }

                    def emit_s(i, group):
                        # one [128, 2*QC] PSUM tile for the pair's S tiles
                        pss = ps_s.tile([128, 2 * QC], fp32, tag="pss",
                                        name=f"pss{ctx['h']}_{i}")
                        out = []
                        lo, hi = None, None
                        for j, live in group:
                            jj = j - jpair[0]
                            t0, t1 = live[0], live[-1]
                            w0, w1 = t0 * 128, (t1 + 1) * 128
                            nc.tensor.matmul(
                                pss[:, jj * QC + w0:jj * QC + w1],
                                KTa[:, ctx["h"], i * 128:(i + 1) * 128],
                                QTa[:, ctx["h"],
                                    j * QC + w0:j * QC + w1],
                                start=True, stop=True)
                            if lo is None:
                                lo = jj * QC + w0
                            hi = jj * QC + w1
                            out.append((j, jj, w0, w1, live))
                        P = ppool.tile([128, 2 * QC], bf16, tag="p",
                                       name=f"p{ctx['h']}_{i}")
                        # single exp over the pair's contiguous live span
                        nc.scalar.activation(P[:, lo:hi], pss[:, lo:hi],
                                             EXP, scale=SCALE)
                        res = []
                        # pair-wide add only when live spans are contiguous
                        all_add = all(
                            out[k][1] * QC + out[k][3] ==
                            out[k + 1][1] * QC + out[k + 1][2]
                            for k in range(len(out) - 1))
                        for j, jj, w0, w1, live in out:
                            first = (ctx["first_i"][j] == i)
                            if first:
                                all_add = False
                            if w0 > 0 and first:
                                nc.vector.memset(P[:, jj * QC:jj * QC + w0],
                                                 0.0)
                            if w1 < QC and first:
                                nc.vector.memset(
                                    P[:, jj * QC + w1:(jj + 1) * QC], 0.0)
                            for t in range(live[0], live[-1] + 1):
                                qb = 4 * j + t
                                base = jj * QC + t * 128
                                if kind[qb][i] == 0:
                                    nc.vector.memset(
                                        P[:, base:base + 128], 0.0)
                                elif kind[qb][i] == 2:
                                    pat = block_pat[(qb, i)]
                                    nc.vector.tensor_mul(
                                        P[:, base:base + 128],
                                        P[:, base:base + 128],
                                        cst[:, pat * 128:(pat + 1) * 128])
                            res.append((j, jj, P, w0, first))
                        # running softmax-denominator accumulation (DVE),
                        # one pair-wide op when possible
                        pacc = ctx["pacc"]
                        if all_add:
                            alo = min(jj * QC + w0
                                      for j, jj, w0, w1, live in out)
                            nc.vector.tensor_add(
                                pacc[:, alo:hi], pacc[:, alo:hi],
                                P[:, alo:hi])
                        else:
                            for j, jj, w0, w1, live in out:
                                if ctx["first_i"][j] == i:
                                    if i == pair_first and jj * QC > 0 \
                                            and j == out[0][0]:
                                        nc.vector.memset(
                                            pacc[:, 0:jj * QC], 0.0)
                                    nc.vector.tensor_copy(
                                        pacc[:, jj * QC:(jj + 1) * QC],
                                        P[:, jj * QC:(jj + 1) * QC])
                                else:
                                    nc.vector.tensor_add(
                                        pacc[:, w0 + jj * QC:
                                             (jj + 1) * QC],
                                        pacc[:, w0 + jj * QC:
                                             (jj + 1) * QC],
                                        P[:, w0 + jj * QC:(jj + 1) * QC])
                        return res

                    prev = None
                    for i in sorted(blocks_j):
                        cur = (ctx, i, emit_s(i, blocks_j[i]))
                        tick_norms()
                        flush_pending()
                        if prev is not None:
                            emit_ovr(*prev)
                        prev = cur
                        if jpair[0] == 2:
                            it23 += 1
                            if it23 >= 6 and (it23 - 6) % 7 == 0 \
                                    and op_units:
                                op_units.pop(0)()
                    if prev is not None:
                        pending.append(prev)
            flush_pending(final=True)
            # leftover chunk-0/1 units, then the chunk-2/3 projection
            for u in op_units:
                u()
            for j in (2, 3):
                for op in range(H // 256):
                    op_unit(j, op)()


def _prep_inputs(x, mask, Wq, Wk, Wv, Wo, patterns):
    import ml_dtypes
    bf16 = ml_dtypes.bfloat16

    # RoPE tables, d-major [HD, L]
    inv_freq = 1.0 / (ROPE_BASE ** (np.arange(0, HD, 2, dtype=np.float64)
                                    / HD))
    t = np.arange(L, dtype=np.float64)
    freqs = np.outer(t, inv_freq)                     # [L, HD/2]
    emb = np.concatenate((freqs, freqs), axis=-1)     # [L, HD]
    cos = np.cos(emb).T.astype(np.float32)            # [HD, L]
    sin = np.sin(emb).T.astype(np.float32)
    sinm = sin.copy()
    sinm[0:64] = -sin[0:64]
    cos_b = cos.astype(bf16)
    sinm_b = sinm.astype(bf16)

    npat = max(len(patterns), 1)
    maskd = np.zeros((npat, 128, 128), dtype=bf16)
    for i, p in enumerate(patterns):
        maskd[i] = p.astype(np.float32).astype(bf16)

    def wprep(wT):
        # [K, M] (contraction-major) -> [128, K//128, M] partition-major
        # (fat DMA packets)
        wT = np.ascontiguousarray(wT).astype(bf16)
        k, m = wT.shape
        return np.ascontiguousarray(
            wT.reshape(k // 128, 128, m).transpose(1, 0, 2))

    NQCl = L // 512
    in_maps = []
    for c in range(NCORES):
        b, half = c // 2, c % 2
        rows = slice(half * HPC * HD, (half + 1) * HPC * HD)
        xT = np.ascontiguousarray(x[b].T).astype(bf16)    # [H, L]
        # [NQC, 128, (H//128)*512]: per chunk, per partition, all 16
        # feature blocks contiguous
        xP = np.ascontiguousarray(
            xT.reshape(H // 128, 128, NQCl, 512)
            .transpose(2, 1, 0, 3)
            .reshape(NQCl, 128, (H // 128) * 512))
        in_maps.append({
            "xP": xP,
            "wqT": wprep(Wq[rows, :].T),
            "wkT": wprep(Wk[rows, :].T),
            "wvT": wprep(Wv[rows, :].T),
            "woT": wprep(Wo[:, rows].T),
            "cosd": cos_b,
            "sinmd": sinm_b,
            "maskd": maskd,
        })
    return in_maps


def kernel(x, mask, Wq, Wk, Wv, Wo, _trace=False):
    from concourse.bass_utils import run_bass_kernel_spmd

    x = np.asarray(x, dtype=np.float32)
    mask2d = np.asarray(mask, dtype=np.int32).reshape(L, L)
    key = mask2d.tobytes()
    if key not in _cache:
        kind, patterns, block_pat = _analyze_mask(mask2d)
        nc = _build(kind, block_pat, len(patterns))
        _cache[key] = (nc, patterns)
    nc, patterns = _cache[key]

    in_maps = _prep_inputs(x, mask, np.asarray(Wq, np.float32),
                           np.asarray(Wk, np.float32),
                           np.asarray(Wv, np.float32),
                           np.asarray(Wo, np.float32), patterns)
    res = run_bass_kernel_spmd(nc, in_maps, list(range(NCORES)),
                               trace=_trace)
    y = np.empty((B, L, H), dtype=np.float32)
    for b in range(B):
        acc = res.results[2 * b]["yT"].astype(np.float32) + \
              res.results[2 * b + 1]["yT"].astype(np.float32)
        y[b] = acc.T
    if _trace:
        kernel.last_results = res
    return y


if __name__ == "__main__":
    import reference
    inputs = reference.setup_inputs()
    inputs = {k: np.asarray(v) for k, v in inputs.items()}
    out = kernel(**inputs)
    exp = np.asarray(reference.reference(**{k: v for k, v in inputs.items()}))
    err = np.abs(out - exp).max() / np.abs(exp).max()
    print("rel err (absmax):", err)



# revision 39
# speedup vs baseline: 1.0711x; 1.0711x over previous
"""Trainium2 Bass kernel: causal multi-head attention with RoPE.

Model: B=4, L=2048, H=2048, NH=16 heads, head_dim=128.
  q = x @ Wq.T ; k = x @ Wk.T ; v = x @ Wv.T        (per-head split)
  q, k <- RoPE(q, k)
  attn = softmax(mask(q k^T / sqrt(hd)))
  out  = (attn @ v) heads-concat @ Wo.T

Sharding (8 cores): hybrid batch x tensor-parallel.  Core c handles
batch b = c//2 and heads half*8..half*8+7 with half = c%2.  Wq/Wk/Wv are
column-sharded (8 heads per core), Wo row-sharded; each core produces a
partial y[b] and the host sums the two partials per batch (the unshard
step) and concatenates batches.

Per-core dataflow (all SBUF-resident, bf16 inputs / fp32 accumulation):
  phase A: Q^T, K^T  [128d x 2048pos] per head (d-major) and V
           [128pos x 1024d] pos-major, via PE matmuls; RoPE on Q^T/K^T
           (rotate-half partition shuffle via SBUF->SBUF DMA, the
           elementwise part on DVE).
  phase B: flash-style causal attention per (head, 512-wide q chunk):
           S^T tile = K_blk^T Q_chunk (PE), P = exp(S^T/sqrt(d)) (ACT),
           block-sparse causal structure with a triangular-mask multiply
           on diagonal blocks (DVE), O^T += V_blk P (PE), rowsum via
           ones-matmul (PE), reciprocal+broadcast+scale for the softmax
           normalization (DVE + GPSIMD).
  phase C: y^T partial = Wo_shard O^T (PE) -> DRAM fp32.
"""

import math
import numpy as np

B, L, H, NH, HD = 4, 2048, 2048, 16, 128
ROPE_BASE = 10000.0
NCORES = 8
HPC = 8          # heads per core
QC = 512         # q chunk width
NQC = L // QC    # 4 q chunks
NKB = L // 128   # 16 kp blocks
SCALE = 1.0 / math.sqrt(HD)

_cache = {}


def _analyze_mask(mask2d):
    """Classify each (q_block, kp_block) 128x128 block of the [L, L] mask.

    Returns (block_kind[16][16] with 0=empty,1=full,2=mixed, patterns,
    pattern_idx dict keyed by block coords). mask2d is int32 [L, L],
    rows=q, cols=kp.
    """
    nb = L // 128
    kind = [[0] * nb for _ in range(nb)]
    patterns = []
    pat_key_to_idx = {}
    block_pat = {}
    for qb in range(nb):
        rows = mask2d[qb * 128:(qb + 1) * 128]
        for kb in range(nb):
            blk = rows[:, kb * 128:(kb + 1) * 128]
            s = int(blk.sum())
            if s == 0:
                kind[qb][kb] = 0
            elif s == 128 * 128:
                kind[qb][kb] = 1
            else:
                kind[qb][kb] = 2
                key = blk.tobytes()
                idx = pat_key_to_idx.get(key)
                if idx is None:
                    idx = len(patterns)
                    pat_key_to_idx[key] = idx
                    # stored transposed: S^T tiles are [kp, q]
                    patterns.append(np.ascontiguousarray(blk.T))
                block_pat[(qb, kb)] = idx
    return kind, patterns, block_pat


def _build(kind, block_pat, n_patterns):
    """Build the SPMD bass program (same for all 8 cores)."""
    import concourse.bass as bass
    import concourse.bacc as bacc
    import concourse.mybir as mybir
    import concourse.tile as tile

    fp32 = mybir.dt.float32
    bf16 = mybir.dt.bfloat16
    EXP = mybir.ActivationFunctionType.Exp

    nc = bacc.Bacc("TRN2", target_bir_lowering=False, debug=False)

    NHC = H // 128  # 16 input-feature blocks

    # All big operands are pre-shuffled on the host into partition-major
    # layouts with LONG contiguous per-partition runs so DMA packets are
    # 8-32KB instead of 1-2KB (per-queue DMA throughput is packet-rate
    # limited).
    xP = nc.dram_tensor("xP", [NQC, 128, NHC * QC], bf16,
                        kind="ExternalInput")
    wqT = nc.dram_tensor("wqT", [128, NHC, HPC * HD], bf16,
                         kind="ExternalInput")
    wkT = nc.dram_tensor("wkT", [128, NHC, HPC * HD], bf16,
                         kind="ExternalInput")
    wvT = nc.dram_tensor("wvT", [128, NHC, HPC * HD], bf16,
                         kind="ExternalInput")
    woT = nc.dram_tensor("woT", [128, HPC, H], bf16, kind="ExternalInput")
    cosd = nc.dram_tensor("cosd", [HD, L], bf16, kind="ExternalInput")
    sinmd = nc.dram_tensor("sinmd", [HD, L], bf16, kind="ExternalInput")
    npat = max(n_patterns, 1)
    maskd = nc.dram_tensor("maskd", [npat, 128, 128], bf16, kind="ExternalInput")
    yT = nc.dram_tensor("yT", [H, L], bf16, kind="ExternalOutput")

    def qk_phase(tc, w_dram, out_a, wpool, xpool, tpool, pspool, wtag,
                 cos_sb, sinm_sb, delay_after=None, after_x0=None):
        """Q^T / K^T d-major projection + fused RoPE per (head, chunk).

        delay_after: instructions the weight DMA must wait for -- used to
        keep the K-phase weight prefetch off the critical head-of-kernel
        DMA bandwidth.  Weight DMAs ride the scalar HWDGE queue so they
        run concurrently with x DMAs on the sync queue.
        """
        from concourse.tile import add_dep_helper
        w_sb = wpool.tile([128, NHC, HPC * HD], bf16, tag="w",
                          name=f"w_{wtag}")
        w_insts = []
        for g in range(4):
            w_insts.append(
                nc.scalar.dma_start(out=w_sb[:, 4 * g:4 * g + 4, :],
                                    in_=w_dram[:, 4 * g:4 * g + 4, :]))
        if delay_after:
            for wi in w_insts:
                for di in delay_after:
                    add_dep_helper(wi.ins, di.ins, reason="defer weight prefetch")
        x0_insts = []
        # RoPE for head h is emitted one head late so the DVE queue (strict
        # 8-deep FIFO) never head-blocks on the rotate-half DMA latency.
        rope_q = []

        def emit_rope(h, js):
            q = out_a[:, h, js]
            rq = tpool.tile([128, QC], bf16, tag="rotq")
            eng = nc.sync if h % 2 == 0 else nc.scalar
            eng.dma_start(out=rq[0:64, :], in_=out_a[64:128, h, js])
            eng.dma_start(out=rq[64:128, :], in_=out_a[0:64, h, js])

            def fire():
                nc.vector.tensor_mul(rq[:], rq[:], sinm_sb[:, js])
                nc.vector.tensor_mul(q, q, cos_sb[:, js])
                nc.vector.tensor_add(q, q, rq[:])
            rope_q.append(fire)
            if len(rope_q) > 1:
                rope_q.pop(0)()

        for j in range(NQC):
            js = slice(j * QC, (j + 1) * QC)
            x_sb = xpool.tile([128, NHC, QC], bf16, tag="xcols",
                              name=f"x_{wtag}{j}")
            for g in range(4):
                di = nc.sync.dma_start(
                    out=x_sb[:, 4 * g:4 * g + 4, :],
                    in_=xP[j, :, 4 * g * QC:(4 * g + 4) * QC])
                if j == 0:
                    x0_insts.append(di)
            if j == 0 and after_x0 is not None:
                after_x0()
            for h in range(HPC):
                ps = pspool.tile([128, QC], fp32, tag="ps_proj")
                for hc in range(NHC):
                    nc.tensor.matmul(
                        ps[:],
                        w_sb[:, hc, h * HD:(h + 1) * HD],
                        x_sb[:, hc, :],
                        start=(hc == 0), stop=(hc == NHC - 1))
                nc.vector.tensor_copy(out_a[:, h, js], ps[:])
                emit_rope(h, js)
        while rope_q:
            rope_q.pop(0)()
        return x0_insts

    def v_phase(tc, w_dram, va, wpool, xpool, pspool, delay_after=None):
        """V pos-major projection (same pre-shuffled x chunks as Q/K)."""
        from concourse.tile import add_dep_helper
        w_sb = wpool.tile([128, NHC, HPC * HD], bf16, tag="w", name="w_v")
        for g in range(4):
            nc.scalar.dma_start(out=w_sb[:, 4 * g:4 * g + 4, :],
                                in_=w_dram[:, 4 * g:4 * g + 4, :])
        for j in range(NQC):
            x_sb = xpool.tile([128, NHC, QC], bf16, tag="xv", name=f"xv{j}")
            for g in range(4):
                di = nc.sync.dma_start(
                    out=x_sb[:, 4 * g:4 * g + 4, :],
                    in_=xP[j, :, 4 * g * QC:(4 * g + 4) * QC])
                if delay_after and j == 0:
                    for d0 in delay_after:
                        add_dep_helper(di.ins, d0.ins,
                                       reason="defer xv prefetch")
            for pb in range(QC // 128):
                psd = [pspool.tile([128, QC], fp32, tag="ps_proj",
                                   name=f"psv{j}_{pb}_{dc}")
                       for dc in range(2)]
                for hc in range(NHC):
                    for dc in range(2):
                        nc.tensor.matmul(
                            psd[dc][:],
                            x_sb[:, hc, pb * 128:(pb + 1) * 128],
                            w_sb[:, hc, dc * QC:(dc + 1) * QC],
                            start=(hc == 0), stop=(hc == NHC - 1))
                for dc in range(2):
                    nc.vector.tensor_copy(
                        va[:, j * (QC // 128) + pb, dc * QC:(dc + 1) * QC],
                        psd[dc][:])

    with tile.TileContext(nc) as tc:
        with tc.tile_pool(name="persist", bufs=1, side="left") as persist:
            # one combined small-constant tile: [trimask patterns | ones]
            cst = persist.tile([128, npat * 128 + 128], bf16, tag="cst")
            ones_col = npat * 128
            nc.vector.memset(cst[:, ones_col:ones_col + 128], 1.0)
            onesf = persist.tile([128, 128], fp32, tag="onesf")
            nc.vector.memset(onesf[:], 1.0)
            # warm the gpsimd partition-ops library now (PE is idle waiting
            # on DMAs anyway); the first real partition_broadcast otherwise
            # stalls attention ~9us on LOAD_LIB.
            nc.gpsimd.partition_broadcast(onesf[:, 0:8], onesf[0:1, 0:8])
            QTa = persist.tile([HD, HPC, L], bf16, tag="qta")
            KTa = persist.tile([HD, HPC, L], bf16, tag="kta")

            # ---------------- phase A: projections + RoPE ----------------
            # Manual pool lifetimes (non-LIFO): weights/x/rope tables are
            # freed before attention while Va spans V-phase..attention.
            wpool_cm = tc.tile_pool(name="wpool", bufs=2, side="right")
            wpool = wpool_cm.__enter__()
            ropec_cm = tc.tile_pool(name="ropec", bufs=1, side="right")
            ropec = ropec_cm.__enter__()
            psp_cm = tc.tile_pool(name="ps_proj", bufs=4, space="PSUM")
            psp = psp_cm.__enter__()

            cos_sb = ropec.tile([HD, L], bf16, tag="cos")
            sinm_sb = ropec.tile([HD, L], bf16, tag="sinm")

            def rope_dma():
                # emitted after the first x chunk so the critical-path
                # startup DMAs (x g0 + wq g0) own the HBM bandwidth
                nc.sync.dma_start(out=cos_sb[:], in_=cosd[:])
                nc.sync.dma_start(out=sinm_sb[:], in_=sinmd[:])

            xv_cm = tc.tile_pool(name="xv", bufs=2, side="right")
            xv = xv_cm.__enter__()
            xqk_cm = tc.tile_pool(name="xqk", bufs=2, side="right")
            xqk = xqk_cm.__enter__()
            tpool_cm = tc.tile_pool(name="tpool", bufs=2, side="right")
            tpool = tpool_cm.__enter__()
            q_x0 = qk_phase(tc, wqT, QTa, wpool, xqk, tpool, psp, "q",
                            cos_sb, sinm_sb, after_x0=rope_dma)
            qk_phase(tc, wkT, KTa, wpool, xqk, tpool, psp, "k",
                     cos_sb, sinm_sb, delay_after=q_x0)
            # tri-mask patterns: needed only in attention; scalar queue
            # behind the wq/wk weight loads
            for p in range(n_patterns):
                nc.scalar.dma_start(out=cst[:, p * 128:(p + 1) * 128],
                                    in_=maskd[p])
            tpool_cm.__exit__(None, None, None)
            xqk_cm.__exit__(None, None, None)
            vp_cm = tc.tile_pool(name="vp", bufs=1, side="left")
            vp_outer = vp_cm.__enter__()
            Va = vp_outer.tile([128, NKB, HPC * HD], bf16, tag="va")
            v_phase(tc, wvT, Va, wpool, xv, psp, delay_after=q_x0)
            xv_cm.__exit__(None, None, None)
            ropec_cm.__exit__(None, None, None)
            wpool_cm.__exit__(None, None, None)
            psp_cm.__exit__(None, None, None)

            # -------- phase B + C under Va's lifetime --------
            _attn_and_out(tc, nc, kind, block_pat, QTa, KTa, Va,
                          cst, ones_col, onesf, woT, yT, fp32, bf16, EXP)
            vp_cm.__exit__(None, None, None)

    nc.compile()
    return nc


def _attn_and_out(tc, nc, kind, block_pat, QTa, KTa, Va, cst, ones_col,
                  onesf, woT, yT, fp32, bf16, EXP):
    ones_sb = cst[:, ones_col:ones_col + 1]
    with tc.tile_pool(name="otp", bufs=1, side="left") as otp, \
         tc.tile_pool(name="wo", bufs=1, side="left") as wop:
        OTa = otp.tile([HD, HPC, L], bf16, tag="ota")
        wo_sb = wop.tile([128, HPC, H], bf16, tag="wo")
        # prefetch Wo during attention, split across both HWDGE queues
        nc.scalar.dma_start(out=wo_sb[:, 0:4, :], in_=woT[:, 0:4, :])
        nc.sync.dma_start(out=wo_sb[:, 4:8, :], in_=woT[:, 4:8, :])

        # ---------------- phase B: attention ----------------
        # q-chunk PAIRS inside the kp-block loop.  Per (i, jpair) both S
        # tiles land in one 2-bank PSUM tile so a single ACT exp covers
        # them (ACT cost is (N+~310)/1.2 ns -- instruction count matters);
        # the O accumulators and the running softmax-denominator Pacc are
        # pair-wide too.  The denominator is finished by a gpsimd
        # partition_all_reduce (cross-partition sum, idle engine) followed
        # by a DVE reciprocal, and the normalization multiplies PSUM O by
        # the all-reduced reciprocal directly -- no ones-matmul, no
        # partition_broadcast, no PSUM rowsum banks.  The last O/normalize
        # group of each pair is deferred into the next pair's instruction
        # stream (software pipeline) so the PE never head-of-line blocks.
        import concourse.bass_isa as bass_isa
        with tc.tile_pool(name="pp", bufs=4, side="right") as ppool, \
             tc.tile_pool(name="aa", bufs=2, side="right") as apool, \
             tc.tile_pool(name="rs", bufs=2, side="right") as rspool, \
             tc.tile_pool(name="ysb", bufs=4, side="right") as ypool, \
             tc.tile_pool(name="ps_s", bufs=2, space="PSUM") as ps_s, \
             tc.tile_pool(name="ps_o", bufs=1, space="PSUM") as ps_o, \
             tc.tile_pool(name="ps_r", bufs=2, space="PSUM") as ps_r:
            pending = []   # deferred last-O emissions (flushed next iter)
            norm_q = []    # deferred recip+normalize closures, with age

            # ---- output-projection units (phase C, interleavable) ----
            # one unit = (q-chunk j, pair of output blocks): 16 matmuls
            # into a 2-bank PSUM tile from the shared pss ring, one ACT
            # copy to SBUF, one fat DMA out.  Chunk 0/1 units interleave
            # into the (2,3)-pair attention stream; chunk 2/3 units run
            # after attention.
            def op_unit(j, op):
                def emit():
                    ps = ps_s.tile([128, 2 * QC], fp32, tag="pss",
                                   name=f"psc{j}_{op}")
                    for koc in range(2):
                        oc = 2 * op + koc
                        for fc in range(HPC):
                            nc.tensor.matmul(
                                ps[:, koc * QC:(koc + 1) * QC],
                                wo_sb[:, fc, oc * 128:(oc + 1) * 128],
                                OTa[:, fc, j * QC:(j + 1) * QC],
                                start=(fc == 0), stop=(fc == HPC - 1))
                    y_sb = ypool.tile([128, 2 * QC], bf16, tag="y")
                    nc.scalar.copy(y_sb[:], ps[:])
                    yr = yT[2 * op * 128:(2 * op + 2) * 128,
                            j * QC:(j + 1) * QC].rearrange(
                                "(a p) m -> p a m", p=128)
                    eng = nc.sync if op % 2 == 0 else nc.scalar
                    eng.dma_start(out=yr, in_=y_sb[:])
                return emit

            op_units = [op_unit(j, op) for j in (0, 1)
                        for op in range(H // 256)]

            def emit_ovr(ctx, i, group):
                h = ctx["h"]
                # O matmuls first (V stationary shared across the pair)
                for j, jj, P, w0, first in group:
                    m0 = 0 if first else w0
                    nc.tensor.matmul(
                        ctx["pso"][:, jj * QC + m0:(jj + 1) * QC],
                        Va[:, i, h * HD:(h + 1) * HD],
                        P[:, jj * QC + m0:(jj + 1) * QC],
                        start=first, stop=(ctx["last_i"][j] == i))
                if i == ctx["pair_last"]:
                    # evacuate O to SBUF on ACT right away: frees the pso
                    # PSUM tile so the next pairs never wait on the
                    # normalization chain
                    oev = rspool.tile([128, 2 * QC], bf16, tag="oev",
                                      name=f"oev{h}_{ctx['jp0']}")
                    nc.scalar.copy(oev[:], ctx["pso"][:])
                    # denominators: two cheap ones-matmuls over the
                    # accumulated Pacc halves (contraction over kp)
                    psr = {}
                    for jj in range(2):
                        psr[jj] = ps_r.tile([1, QC], fp32, tag="psr",
                                            name=f"psr{h}_{ctx['jp0']}{jj}")
                        nc.tensor.matmul(
                            psr[jj][0:1, :], ones_sb,
                            ctx["pacc"][:, jj * QC:(jj + 1) * QC],
                            start=True, stop=True)

                    def norm(ctx=ctx, psr=psr, oev=oev, h=h):
                        rp = rspool.tile([1, 2 * QC], fp32, tag="rp",
                                         name=f"rp{h}_{ctx['jp0']}")
                        for jj in range(2):
                            nc.vector.reciprocal_approx_fast(
                                out=rp[0:1, jj * QC:(jj + 1) * QC],
                                in_=psr[jj][0:1, :])
                        rb = rspool.tile([1, 2 * QC], bf16, tag="rb",
                                         name=f"rb{h}_{ctx['jp0']}")
                        nc.vector.tensor_copy(rb[0:1, :], rp[0:1, :])
                        bc = rspool.tile([128, 2 * QC], bf16, tag="bc",
                                         name=f"bc{h}_{ctx['jp0']}")
                        nc.gpsimd.partition_broadcast(bc[:], rb[0:1, :])
                        nc.vector.tensor_mul(
                            OTa[:, h,
                                ctx["jp0"] * QC:(ctx["jp0"] + 2) * QC],
                            oev[:], bc[:])
                    norm_q.append([0, norm])

            def tick_norms(final=False):
                for e in norm_q:
                    e[0] += 1
                while norm_q and (final or norm_q[0][0] >= 3):
                    norm_q.pop(0)[1]()

            def flush_pending(final=False):
                while pending:
                    emit_ovr(*pending.pop(0))
                if final:
                    tick_norms(final=True)

            it23 = 0
            for jpair, h in [((0, 1), hh) for hh in range(HPC)] + \
                            [((2, 3), hh) for hh in range(HPC)]:
                if True:
                    blocks_j = {}
                    first_i = {}
                    last_i = {}
                    for j in jpair:
                        for i in range(NKB):
                            live = [t for t in range(4)
                                    if kind[4 * j + t][i] != 0]
                            if live:
                                blocks_j.setdefault(i, []).append((j, live))
                                if j not in first_i:
                                    first_i[j] = i
                                last_i[j] = i
                    if not first_i:
                        continue
                    pair_first = min(first_i.values())
                    ctx = {
                        "h": h,
                        "jp0": jpair[0],
                        "first_i": first_i,
                        "last_i": last_i,
                        "pair_last": max(last_i.values()),
                        "pso": ps_o.tile([128, 2 * QC], fp32, tag="pso",
                                         name=f"pso{h}_{jpair[0]}"),
                        "pacc": apool.tile([128, 2 * QC], bf16, tag="pacc",
                                           name=f"pacc{h}_{jpair[0]}"),
                    }

                    def emit_s(i, group):
                        # one [128, 2*QC] PSUM tile for the pair's S tiles
                        pss = ps_s.tile([128, 2 * QC], fp32, tag="pss",
                                        name=f"pss{ctx['h']}_{i}")
                        out = []
                        lo, hi = None, None
                        for j, live in group:
                            jj = j - jpair[0]
                            t0, t1 = live[0], live[-1]
                            w0, w1 = t0 * 128, (t1 + 1) * 128
                            nc.tensor.matmul(
                                pss[:, jj * QC + w0:jj * QC + w1],
                                KTa[:, ctx["h"], i * 128:(i + 1) * 128],
                                QTa[:, ctx["h"],
                                    j * QC + w0:j * QC + w1],
                                start=True, stop=True)
                            if lo is None:
                                lo = jj * QC + w0
                            hi = jj * QC + w1
                            out.append((j, jj, w0, w1, live))
                        P = ppool.tile([128, 2 * QC], bf16, tag="p",
                                       name=f"p{ctx['h']}_{i}")
                        # single exp over the pair's contiguous live span
                        nc.scalar.activation(P[:, lo:hi], pss[:, lo:hi],
                                             EXP, scale=SCALE)
                        res = []
                        # pair-wide add only when live spans are contiguous
                        all_add = all(
                            out[k][1] * QC + out[k][3] ==
                            out[k + 1][1] * QC + out[k + 1][2]
                            for k in range(len(out) - 1))
                        for j, jj, w0, w1, live in out:
                            first = (ctx["first_i"][j] == i)
                            if first:
                                all_add = False
                            if w0 > 0 and first:
                                nc.vector.memset(P[:, jj * QC:jj * QC + w0],
                                                 0.0)
                            if w1 < QC and first:
                                nc.vector.memset(
                                    P[:, jj * QC + w1:(jj + 1) * QC], 0.0)
                            for t in range(live[0], live[-1] + 1):
                                qb = 4 * j + t
                                base = jj * QC + t * 128
                                if kind[qb][i] == 0:
                                    nc.vector.memset(
                                        P[:, base:base + 128], 0.0)
                                elif kind[qb][i] == 2:
                                    pat = block_pat[(qb, i)]
                                    nc.vector.tensor_mul(
                                        P[:, base:base + 128],
                                        P[:, base:base + 128],
                                        cst[:, pat * 128:(pat + 1) * 128])
                            res.append((j, jj, P, w0, first))
                        # running softmax-denominator accumulation (DVE),
                        # one pair-wide op when possible
                        pacc = ctx["pacc"]
                        if all_add:
                            alo = min(jj * QC + w0
                                      for j, jj, w0, w1, live in out)
                            nc.vector.tensor_add(
                                pacc[:, alo:hi], pacc[:, alo:hi],
                                P[:, alo:hi])
                        else:
                            for j, jj, w0, w1, live in out:
                                if ctx["first_i"][j] == i:
                                    if i == pair_first and jj * QC > 0 \
                                            and j == out[0][0]:
                                        nc.vector.memset(
                                            pacc[:, 0:jj * QC], 0.0)
                                    nc.vector.tensor_copy(
                                        pacc[:, jj * QC:(jj + 1) * QC],
                                        P[:, jj * QC:(jj + 1) * QC])
                                else:
                                    nc.vector.tensor_add(
                                        pacc[:, w0 + jj * QC:
                                             (jj + 1) * QC],
                                        pacc[:, w0 + jj * QC:
                                             (jj + 1) * QC],
                                        P[:, w0 + jj * QC:(jj + 1) * QC])
                        return res

                    prev = None
                    for i in sorted(blocks_j):
                        cur = (ctx, i, emit_s(i, blocks_j[i]))
                        tick_norms()
                        flush_pending()
                        if prev is not None:
                            emit_ovr(*prev)
                        prev = cur
                        if jpair[0] == 2:
                            it23 += 1
                            if it23 >= 6 and (it23 - 6) % 7 == 0 \
                                    and op_units:
                                op_units.pop(0)()
                    if prev is not None:
                        pending.append(prev)
            flush_pending(final=True)
            # leftover chunk-0/1 units, then the chunk-2/3 projection
            for u in op_units:
                u()
            for j in (2, 3):
                for op in range(H // 256):
                    op_unit(j, op)()


def _prep_inputs(x, mask, Wq, Wk, Wv, Wo, patterns):
    import ml_dtypes
    bf16 = ml_dtypes.bfloat16

    # RoPE tables, d-major [HD, L]
    inv_freq = 1.0 / (ROPE_BASE ** (np.arange(0, HD, 2, dtype=np.float64)
                                    / HD))
    t = np.arange(L, dtype=np.float64)
    freqs = np.outer(t, inv_freq)                     # [L, HD/2]
    emb = np.concatenate((freqs, freqs), axis=-1)     # [L, HD]
    cos = np.cos(emb).T.astype(np.float32)            # [HD, L]
    sin = np.sin(emb).T.astype(np.float32)
    sinm = sin.copy()
    sinm[0:64] = -sin[0:64]
    cos_b = cos.astype(bf16)
    sinm_b = sinm.astype(bf16)

    npat = max(len(patterns), 1)
    maskd = np.zeros((npat, 128, 128), dtype=bf16)
    for i, p in enumerate(patterns):
        maskd[i] = p.astype(np.float32).astype(bf16)

    def wprep(wT):
        # [K, M] (contraction-major) -> [128, K//128, M] partition-major
        # (fat DMA packets)
        wT = np.ascontiguousarray(wT).astype(bf16)
        k, m = wT.shape
        return np.ascontiguousarray(
            wT.reshape(k // 128, 128, m).transpose(1, 0, 2))

    NQCl = L // 512
    in_maps = []
    for c in range(NCORES):
        b, half = c // 2, c % 2
        rows = slice(half * HPC * HD, (half + 1) * HPC * HD)
        xT = np.ascontiguousarray(x[b].T).astype(bf16)    # [H, L]
        # [NQC, 128, (H//128)*512]: per chunk, per partition, all 16
        # feature blocks contiguous
        xP = np.ascontiguousarray(
            xT.reshape(H // 128, 128, NQCl, 512)
            .transpose(2, 1, 0, 3)
            .reshape(NQCl, 128, (H // 128) * 512))
        in_maps.append({
            "xP": xP,
            "wqT": wprep(Wq[rows, :].T),
            "wkT": wprep(Wk[rows, :].T),
            "wvT": wprep(Wv[rows, :].T),
            "woT": wprep(Wo[:, rows].T),
            "cosd": cos_b,
            "sinmd": sinm_b,
            "maskd": maskd,
        })
    return in_maps


def kernel(x, mask, Wq, Wk, Wv, Wo, _trace=False):
    from concourse.bass_utils import run_bass_kernel_spmd

    x = np.asarray(x, dtype=np.float32)
    mask2d = np.asarray(mask, dtype=np.int32).reshape(L, L)
    key = mask2d.tobytes()
    if key not in _cache:
        kind, patterns, block_pat = _analyze_mask(mask2d)
        nc = _build(kind, block_pat, len(patterns))
        _cache[key] = (nc, patterns)
    nc, patterns = _cache[key]

    in_maps = _prep_inputs(x, mask, np.asarray(Wq, np.float32),
                           np.asarray(Wk, np.float32),
                           np.asarray(Wv, np.float32),
                           np.asarray(Wo, np.float32), patterns)
    res = run_bass_kernel_spmd(nc, in_maps, list(range(NCORES)),
                               trace=_trace)
    y = np.empty((B, L, H), dtype=np.float32)
    for b in range(B):
        acc = res.results[2 * b]["yT"].astype(np.float32) + \
              res.results[2 * b + 1]["yT"].astype(np.float32)
        y[b] = acc.T
    if _trace:
        kernel.last_results = res
    return y


if __name__ == "__main__":
    import reference
    inputs = reference.setup_inputs()
    inputs = {k: np.asarray(v) for k, v in inputs.items()}
    out = kernel(**inputs)
    exp = np.asarray(reference.reference(**{k: v for k, v in inputs.items()}))
    err = np.abs(out - exp).max() / np.abs(exp).max()
    print("rel err (absmax):", err)



# revision 49
# speedup vs baseline: 1.0804x; 1.0087x over previous
"""Trainium2 Bass kernel: causal multi-head attention with RoPE.

Model: B=4, L=2048, H=2048, NH=16 heads, head_dim=128.
  q = x @ Wq.T ; k = x @ Wk.T ; v = x @ Wv.T        (per-head split)
  q, k <- RoPE(q, k)
  attn = softmax(mask(q k^T / sqrt(hd)))
  out  = (attn @ v) heads-concat @ Wo.T

Sharding (8 cores): hybrid batch x tensor-parallel.  Core c handles
batch b = c//2 and heads half*8..half*8+7 with half = c%2.  Wq/Wk/Wv are
column-sharded (8 heads per core), Wo row-sharded; each core produces a
partial y[b] and the host sums the two partials per batch (the unshard
step) and concatenates batches.

Per-core dataflow (all SBUF-resident, bf16 inputs / fp32 accumulation):
  phase A: Q^T, K^T  [128d x 2048pos] per head (d-major) and V
           [128pos x 1024d] pos-major, via PE matmuls; RoPE on Q^T/K^T
           (rotate-half partition shuffle via SBUF->SBUF DMA, the
           elementwise part on DVE).
  phase B: flash-style causal attention per (head, 512-wide q chunk):
           S^T tile = K_blk^T Q_chunk (PE), P = exp(S^T/sqrt(d)) (ACT),
           block-sparse causal structure with a triangular-mask multiply
           on diagonal blocks (DVE), O^T += V_blk P (PE), rowsum via
           ones-matmul (PE), reciprocal+broadcast+scale for the softmax
           normalization (DVE + GPSIMD).
  phase C: y^T partial = Wo_shard O^T (PE) -> DRAM fp32.
"""

import math
import numpy as np

B, L, H, NH, HD = 4, 2048, 2048, 16, 128
ROPE_BASE = 10000.0
NCORES = 8
HPC = 8          # heads per core
QC = 512         # q chunk width
NQC = L // QC    # 4 q chunks
NKB = L // 128   # 16 kp blocks
SCALE = 1.0 / math.sqrt(HD)

_cache = {}


def _analyze_mask(mask2d):
    """Classify each (q_block, kp_block) 128x128 block of the [L, L] mask.

    Returns (block_kind[16][16] with 0=empty,1=full,2=mixed, patterns,
    pattern_idx dict keyed by block coords). mask2d is int32 [L, L],
    rows=q, cols=kp.
    """
    nb = L // 128
    kind = [[0] * nb for _ in range(nb)]
    patterns = []
    pat_key_to_idx = {}
    block_pat = {}
    for qb in range(nb):
        rows = mask2d[qb * 128:(qb + 1) * 128]
        for kb in range(nb):
            blk = rows[:, kb * 128:(kb + 1) * 128]
            s = int(blk.sum())
            if s == 0:
                kind[qb][kb] = 0
            elif s == 128 * 128:
                kind[qb][kb] = 1
            else:
                kind[qb][kb] = 2
                key = blk.tobytes()
                idx = pat_key_to_idx.get(key)
                if idx is None:
                    idx = len(patterns)
                    pat_key_to_idx[key] = idx
                    # stored transposed: S^T tiles are [kp, q]
                    patterns.append(np.ascontiguousarray(blk.T))
                block_pat[(qb, kb)] = idx
    return kind, patterns, block_pat


def _build(kind, block_pat, n_patterns):
    """Build the SPMD bass program (same for all 8 cores)."""
    import concourse.bass as bass
    import concourse.bacc as bacc
    import concourse.mybir as mybir
    import concourse.tile as tile

    fp32 = mybir.dt.float32
    bf16 = mybir.dt.bfloat16
    EXP = mybir.ActivationFunctionType.Exp

    nc = bacc.Bacc("TRN2", target_bir_lowering=False, debug=False)

    NHC = H // 128  # 16 input-feature blocks

    # All big operands are pre-shuffled on the host into partition-major
    # layouts with LONG contiguous per-partition runs so DMA packets are
    # 8-32KB instead of 1-2KB (per-queue DMA throughput is packet-rate
    # limited).
    xP = nc.dram_tensor("xP", [NQC, 128, NHC * QC], bf16,
                        kind="ExternalInput")
    wqT = nc.dram_tensor("wqT", [128, NHC, HPC * HD], bf16,
                         kind="ExternalInput")
    wkT = nc.dram_tensor("wkT", [128, NHC, HPC * HD], bf16,
                         kind="ExternalInput")
    wvT = nc.dram_tensor("wvT", [128, NHC, HPC * HD], bf16,
                         kind="ExternalInput")
    woT = nc.dram_tensor("woT", [128, HPC, H], bf16, kind="ExternalInput")
    cosd = nc.dram_tensor("cosd", [HD, L], bf16, kind="ExternalInput")
    sinmd = nc.dram_tensor("sinmd", [HD, L], bf16, kind="ExternalInput")
    npat = max(n_patterns, 1)
    maskd = nc.dram_tensor("maskd", [npat, 128, 128], bf16, kind="ExternalInput")
    yT = nc.dram_tensor("yT", [H, L], bf16, kind="ExternalOutput")

    def qk_phase(tc, w_dram, out_a, wpool, xpool, tpool, pspool, wtag,
                 cos_sb, sinm_sb, delay_after=None, after_x0=None):
        """Q^T / K^T d-major projection + fused RoPE per (head, chunk).

        delay_after: instructions the weight DMA must wait for -- used to
        keep the K-phase weight prefetch off the critical head-of-kernel
        DMA bandwidth.  Weight DMAs ride the scalar HWDGE queue so they
        run concurrently with x DMAs on the sync queue.
        """
        from concourse.tile import add_dep_helper
        w_sb = wpool.tile([128, NHC, HPC * HD], bf16, tag="w",
                          name=f"w_{wtag}")
        w_insts = []
        first = wtag == "q"
        # head-of-kernel: interleave the first weight/x groups across the
        # two HWDGE queues in consumption order so hc-group g's operands
        # land just in time (w g1/g3 are emitted after x j0 below)
        for g in ((0, 2) if first else (0, 1, 2, 3)):
            w_insts.append(
                nc.scalar.dma_start(out=w_sb[:, 4 * g:4 * g + 4, :],
                                    in_=w_dram[:, 4 * g:4 * g + 4, :]))
        if delay_after:
            for wi in w_insts:
                for di in delay_after:
                    add_dep_helper(wi.ins, di.ins, reason="defer weight prefetch")
        x0_insts = []
        # RoPE for head h is emitted one head late so the DVE queue (strict
        # 8-deep FIFO) never head-blocks on the rotate-half DMA latency.
        rope_q = []

        def emit_rope(h, js):
            q = out_a[:, h, js]
            rq = tpool.tile([128, QC], bf16, tag="rotq")
            eng = nc.sync if h % 2 == 0 else nc.scalar
            eng.dma_start(out=rq[0:64, :], in_=out_a[64:128, h, js])
            eng.dma_start(out=rq[64:128, :], in_=out_a[0:64, h, js])

            def fire():
                nc.vector.tensor_mul(rq[:], rq[:], sinm_sb[:, js])
                nc.vector.tensor_mul(q, q, cos_sb[:, js])
                nc.vector.tensor_add(q, q, rq[:])
            rope_q.append(fire)
            if len(rope_q) > 1:
                rope_q.pop(0)()

        for j in range(NQC):
            js = slice(j * QC, (j + 1) * QC)
            x_sb = xpool.tile([128, NHC, QC], bf16, tag="xcols",
                              name=f"x_{wtag}{j}")
            if first and j == 0:
                for g in (0, 2):
                    x0_insts.append(nc.sync.dma_start(
                        out=x_sb[:, 4 * g:4 * g + 4, :],
                        in_=xP[j, :, 4 * g * QC:(4 * g + 4) * QC]))
                for g in (1, 3):
                    x0_insts.append(nc.scalar.dma_start(
                        out=x_sb[:, 4 * g:4 * g + 4, :],
                        in_=xP[j, :, 4 * g * QC:(4 * g + 4) * QC]))
                for g in (1, 3):
                    nc.sync.dma_start(
                        out=w_sb[:, 4 * g:4 * g + 4, :],
                        in_=w_dram[:, 4 * g:4 * g + 4, :])
            else:
                for g in range(4):
                    di = nc.sync.dma_start(
                        out=x_sb[:, 4 * g:4 * g + 4, :],
                        in_=xP[j, :, 4 * g * QC:(4 * g + 4) * QC])
                    if j == 0:
                        x0_insts.append(di)
            if j == 0 and after_x0 is not None:
                after_x0()
            for h in range(HPC):
                ps = pspool.tile([128, QC], fp32, tag="ps_proj")
                for hc in range(NHC):
                    nc.tensor.matmul(
                        ps[:],
                        w_sb[:, hc, h * HD:(h + 1) * HD],
                        x_sb[:, hc, :],
                        start=(hc == 0), stop=(hc == NHC - 1))
                nc.vector.tensor_copy(out_a[:, h, js], ps[:])
                emit_rope(h, js)
        while rope_q:
            rope_q.pop(0)()
        return x0_insts

    def v_phase(tc, w_dram, va, wpool, xpool, pspool, delay_after=None):
        """V pos-major projection (same pre-shuffled x chunks as Q/K)."""
        from concourse.tile import add_dep_helper
        w_sb = wpool.tile([128, NHC, HPC * HD], bf16, tag="w", name="w_v")
        for g in range(4):
            nc.scalar.dma_start(out=w_sb[:, 4 * g:4 * g + 4, :],
                                in_=w_dram[:, 4 * g:4 * g + 4, :])
        for j in range(NQC):
            x_sb = xpool.tile([128, NHC, QC], bf16, tag="xv", name=f"xv{j}")
            for g in range(4):
                di = nc.sync.dma_start(
                    out=x_sb[:, 4 * g:4 * g + 4, :],
                    in_=xP[j, :, 4 * g * QC:(4 * g + 4) * QC])
                if delay_after and j == 0:
                    for d0 in delay_after:
                        add_dep_helper(di.ins, d0.ins,
                                       reason="defer xv prefetch")
            for pb in range(QC // 128):
                psd = [pspool.tile([128, QC], fp32, tag="ps_proj",
                                   name=f"psv{j}_{pb}_{dc}")
                       for dc in range(2)]
                for hc in range(NHC):
                    for dc in range(2):
                        nc.tensor.matmul(
                            psd[dc][:],
                            x_sb[:, hc, pb * 128:(pb + 1) * 128],
                            w_sb[:, hc, dc * QC:(dc + 1) * QC],
                            start=(hc == 0), stop=(hc == NHC - 1))
                for dc in range(2):
                    nc.vector.tensor_copy(
                        va[:, j * (QC // 128) + pb, dc * QC:(dc + 1) * QC],
                        psd[dc][:])

    with tile.TileContext(nc) as tc:
        with tc.tile_pool(name="persist", bufs=1, side="left") as persist:
            # one combined small-constant tile: [trimask patterns | ones]
            cst = persist.tile([128, npat * 128 + 128], bf16, tag="cst")
            ones_col = npat * 128
            nc.vector.memset(cst[:, ones_col:ones_col + 128], 1.0)
            onesf = persist.tile([128, 128], fp32, tag="onesf")
            nc.vector.memset(onesf[:], 1.0)
            # warm the gpsimd partition-ops library now (PE is idle waiting
            # on DMAs anyway); the first real partition_broadcast otherwise
            # stalls attention ~9us on LOAD_LIB.
            nc.gpsimd.partition_broadcast(onesf[:, 0:8], onesf[0:1, 0:8])
            QTa = persist.tile([HD, HPC, L], bf16, tag="qta")
            KTa = persist.tile([HD, HPC, L], bf16, tag="kta")

            # ---------------- phase A: projections + RoPE ----------------
            # Manual pool lifetimes (non-LIFO): weights/x/rope tables are
            # freed before attention while Va spans V-phase..attention.
            wpool_cm = tc.tile_pool(name="wpool", bufs=2, side="right")
            wpool = wpool_cm.__enter__()
            ropec_cm = tc.tile_pool(name="ropec", bufs=1, side="right")
            ropec = ropec_cm.__enter__()
            psp_cm = tc.tile_pool(name="ps_proj", bufs=4, space="PSUM")
            psp = psp_cm.__enter__()

            cos_sb = ropec.tile([HD, L], bf16, tag="cos")
            sinm_sb = ropec.tile([HD, L], bf16, tag="sinm")

            def rope_dma():
                # emitted after the first x chunk so the critical-path
                # startup DMAs (x g0 + wq g0) own the HBM bandwidth
                nc.sync.dma_start(out=cos_sb[:], in_=cosd[:])
                nc.sync.dma_start(out=sinm_sb[:], in_=sinmd[:])

            xv_cm = tc.tile_pool(name="xv", bufs=2, side="right")
            xv = xv_cm.__enter__()
            xqk_cm = tc.tile_pool(name="xqk", bufs=2, side="right")
            xqk = xqk_cm.__enter__()
            tpool_cm = tc.tile_pool(name="tpool", bufs=2, side="right")
            tpool = tpool_cm.__enter__()
            q_x0 = qk_phase(tc, wqT, QTa, wpool, xqk, tpool, psp, "q",
                            cos_sb, sinm_sb, after_x0=rope_dma)
            qk_phase(tc, wkT, KTa, wpool, xqk, tpool, psp, "k",
                     cos_sb, sinm_sb, delay_after=q_x0)
            # tri-mask patterns: needed only in attention; scalar queue
            # behind the wq/wk weight loads
            for p in range(n_patterns):
                nc.scalar.dma_start(out=cst[:, p * 128:(p + 1) * 128],
                                    in_=maskd[p])
            tpool_cm.__exit__(None, None, None)
            xqk_cm.__exit__(None, None, None)
            vp_cm = tc.tile_pool(name="vp", bufs=1, side="left")
            vp_outer = vp_cm.__enter__()
            Va = vp_outer.tile([128, NKB, HPC * HD], bf16, tag="va")
            v_phase(tc, wvT, Va, wpool, xv, psp, delay_after=q_x0)
            xv_cm.__exit__(None, None, None)
            ropec_cm.__exit__(None, None, None)
            wpool_cm.__exit__(None, None, None)
            psp_cm.__exit__(None, None, None)

            # -------- phase B + C under Va's lifetime --------
            _attn_and_out(tc, nc, kind, block_pat, QTa, KTa, Va,
                          cst, ones_col, onesf, woT, yT, fp32, bf16, EXP)
            vp_cm.__exit__(None, None, None)

    nc.compile()
    return nc


def _attn_and_out(tc, nc, kind, block_pat, QTa, KTa, Va, cst, ones_col,
                  onesf, woT, yT, fp32, bf16, EXP):
    ones_sb = cst[:, ones_col:ones_col + 1]
    with tc.tile_pool(name="otp", bufs=1, side="left") as otp, \
         tc.tile_pool(name="wo", bufs=1, side="left") as wop:
        OTa = otp.tile([HD, HPC, L], bf16, tag="ota")
        wo_sb = wop.tile([128, HPC, H], bf16, tag="wo")
        # prefetch Wo during attention, split across both HWDGE queues
        nc.scalar.dma_start(out=wo_sb[:, 0:4, :], in_=woT[:, 0:4, :])
        nc.sync.dma_start(out=wo_sb[:, 4:8, :], in_=woT[:, 4:8, :])

        # ---------------- phase B: attention ----------------
        # q-chunk PAIRS inside the kp-block loop.  Per (i, jpair) both S
        # tiles land in one 2-bank PSUM tile so a single ACT exp covers
        # them (ACT cost is (N+~310)/1.2 ns -- instruction count matters);
        # the O accumulators and the running softmax-denominator Pacc are
        # pair-wide too.  The denominator is finished by a gpsimd
        # partition_all_reduce (cross-partition sum, idle engine) followed
        # by a DVE reciprocal, and the normalization multiplies PSUM O by
        # the all-reduced reciprocal directly -- no ones-matmul, no
        # partition_broadcast, no PSUM rowsum banks.  The last O/normalize
        # group of each pair is deferred into the next pair's instruction
        # stream (software pipeline) so the PE never head-of-line blocks.
        import concourse.bass_isa as bass_isa
        with tc.tile_pool(name="pp", bufs=4, side="right") as ppool, \
             tc.tile_pool(name="aa", bufs=2, side="right") as apool, \
             tc.tile_pool(name="rs", bufs=2, side="right") as rspool, \
             tc.tile_pool(name="ysb", bufs=4, side="right") as ypool, \
             tc.tile_pool(name="ps_s", bufs=2, space="PSUM") as ps_s, \
             tc.tile_pool(name="ps_o", bufs=1, space="PSUM") as ps_o, \
             tc.tile_pool(name="ps_r", bufs=2, space="PSUM") as ps_r:
            pending = []   # deferred last-O emissions (flushed next iter)
            norm_q = []    # deferred recip+normalize closures, with age

            # ---- output-projection units (phase C, interleavable) ----
            # one unit = (q-chunk j, pair of output blocks): 16 matmuls
            # into a 2-bank PSUM tile from the shared pss ring, one ACT
            # copy to SBUF, one fat DMA out.  Chunk 0/1 units interleave
            # into the (2,3)-pair attention stream; chunk 2/3 units run
            # after attention.
            def op_unit(j, op):
                def emit():
                    ps = ps_s.tile([128, 2 * QC], fp32, tag="pss",
                                   name=f"psc{j}_{op}")
                    for koc in range(2):
                        oc = 2 * op + koc
                        for fc in range(HPC):
                            nc.tensor.matmul(
                                ps[:, koc * QC:(koc + 1) * QC],
                                wo_sb[:, fc, oc * 128:(oc + 1) * 128],
                                OTa[:, fc, j * QC:(j + 1) * QC],
                                start=(fc == 0), stop=(fc == HPC - 1))
                    y_sb = ypool.tile([128, 2 * QC], bf16, tag="y")
                    nc.scalar.copy(y_sb[:], ps[:])
                    yr = yT[2 * op * 128:(2 * op + 2) * 128,
                            j * QC:(j + 1) * QC].rearrange(
                                "(a p) m -> p a m", p=128)
                    eng = nc.sync if op % 2 == 0 else nc.scalar
                    eng.dma_start(out=yr, in_=y_sb[:])
                return emit

            op_units = [op_unit(j, op) for j in (0, 1)
                        for op in range(H // 256)]

            def emit_ovr(ctx, i, group):
                h = ctx["h"]
                # O matmuls first (V stationary shared across the pair)
                for j, jj, P, w0, first in group:
                    m0 = 0 if first else w0
                    nc.tensor.matmul(
                        ctx["pso"][:, jj * QC + m0:(jj + 1) * QC],
                        Va[:, i, h * HD:(h + 1) * HD],
                        P[:, jj * QC + m0:(jj + 1) * QC],
                        start=first, stop=(ctx["last_i"][j] == i))
                if i == ctx["pair_last"]:
                    # evacuate O to SBUF on ACT right away: frees the pso
                    # PSUM tile so the next pairs never wait on the
                    # normalization chain
                    oev = rspool.tile([128, 2 * QC], bf16, tag="oev",
                                      name=f"oev{h}_{ctx['jp0']}")
                    nc.scalar.copy(oev[:], ctx["pso"][:])
                    # denominators: two cheap ones-matmuls over the
                    # accumulated Pacc halves (contraction over kp)
                    psr = {}
                    for jj in range(2):
                        psr[jj] = ps_r.tile([1, QC], fp32, tag="psr",
                                            name=f"psr{h}_{ctx['jp0']}{jj}")
                        nc.tensor.matmul(
                            psr[jj][0:1, :], ones_sb,
                            ctx["pacc"][:, jj * QC:(jj + 1) * QC],
                            start=True, stop=True)

                    def norm(ctx=ctx, psr=psr, oev=oev, h=h):
                        rp = rspool.tile([1, 2 * QC], fp32, tag="rp",
                                         name=f"rp{h}_{ctx['jp0']}")
                        for jj in range(2):
                            nc.vector.reciprocal_approx_fast(
                                out=rp[0:1, jj * QC:(jj + 1) * QC],
                                in_=psr[jj][0:1, :])
                        rb = rspool.tile([1, 2 * QC], bf16, tag="rb",
                                         name=f"rb{h}_{ctx['jp0']}")
                        nc.vector.tensor_copy(rb[0:1, :], rp[0:1, :])
                        bc = rspool.tile([128, 2 * QC], bf16, tag="bc",
                                         name=f"bc{h}_{ctx['jp0']}")
                        nc.gpsimd.partition_broadcast(bc[:], rb[0:1, :])
                        nc.vector.tensor_mul(
                            OTa[:, h,
                                ctx["jp0"] * QC:(ctx["jp0"] + 2) * QC],
                            oev[:], bc[:])
                    norm_q.append([0, norm])

            def tick_norms(final=False):
                for e in norm_q:
                    e[0] += 1
                while norm_q and (final or norm_q[0][0] >= 3):
                    norm_q.pop(0)[1]()

            def flush_pending(final=False):
                while pending:
                    emit_ovr(*pending.pop(0))
                if final:
                    tick_norms(final=True)

            it23 = 0
            for jpair, h in [((0, 1), hh) for hh in range(HPC)] + \
                            [((2, 3), hh) for hh in range(HPC)]:
                if True:
                    blocks_j = {}
                    first_i = {}
                    last_i = {}
                    for j in jpair:
                        for i in range(NKB):
                            live = [t for t in range(4)
                                    if kind[4 * j + t][i] != 0]
                            if live:
                                blocks_j.setdefault(i, []).append((j, live))
                                if j not in first_i:
                                    first_i[j] = i
                                last_i[j] = i
                    if not first_i:
                        continue
                    pair_first = min(first_i.values())
                    ctx = {
                        "h": h,
                        "jp0": jpair[0],
                        "first_i": first_i,
                        "last_i": last_i,
                        "pair_last": max(last_i.values()),
                        "pso": ps_o.tile([128, 2 * QC], fp32, tag="pso",
                                         name=f"pso{h}_{jpair[0]}"),
                        "pacc": apool.tile([128, 2 * QC], bf16, tag="pacc",
                                           name=f"pacc{h}_{jpair[0]}"),
                    }

                    def emit_s(i, group):
                        # one [128, 2*QC] PSUM tile for the pair's S tiles
                        pss = ps_s.tile([128, 2 * QC], fp32, tag="pss",
                                        name=f"pss{ctx['h']}_{i}")
                        out = []
                        lo, hi = None, None
                        for j, live in group:
                            jj = j - jpair[0]
                            t0, t1 = live[0], live[-1]
                            w0, w1 = t0 * 128, (t1 + 1) * 128
                            nc.tensor.matmul(
                                pss[:, jj * QC + w0:jj * QC + w1],
                                KTa[:, ctx["h"], i * 128:(i + 1) * 128],
                                QTa[:, ctx["h"],
                                    j * QC + w0:j * QC + w1],
                                start=True, stop=True)
                            if lo is None:
                                lo = jj * QC + w0
                            hi = jj * QC + w1
                            out.append((j, jj, w0, w1, live))
                        P = ppool.tile([128, 2 * QC], bf16, tag="p",
                                       name=f"p{ctx['h']}_{i}")
                        # single exp over the pair's contiguous live span
                        nc.scalar.activation(P[:, lo:hi], pss[:, lo:hi],
                                             EXP, scale=SCALE)
                        res = []
                        # pair-wide add only when live spans are contiguous
                        all_add = all(
                            out[k][1] * QC + out[k][3] ==
                            out[k + 1][1] * QC + out[k + 1][2]
                            for k in range(len(out) - 1))
                        for j, jj, w0, w1, live in out:
                            first = (ctx["first_i"][j] == i)
                            if first:
                                all_add = False
                            if w0 > 0 and first:
                                nc.vector.memset(P[:, jj * QC:jj * QC + w0],
                                                 0.0)
                            if w1 < QC and first:
                                nc.vector.memset(
                                    P[:, jj * QC + w1:(jj + 1) * QC], 0.0)
                            for t in range(live[0], live[-1] + 1):
                                qb = 4 * j + t
                                base = jj * QC + t * 128
                                if kind[qb][i] == 0:
                                    nc.vector.memset(
                                        P[:, base:base + 128], 0.0)
                                elif kind[qb][i] == 2:
                                    pat = block_pat[(qb, i)]
                                    nc.vector.tensor_mul(
                                        P[:, base:base + 128],
                                        P[:, base:base + 128],
                                        cst[:, pat * 128:(pat + 1) * 128])
                            res.append((j, jj, P, w0, first))
                        # running softmax-denominator accumulation (DVE),
                        # one pair-wide op when possible
                        pacc = ctx["pacc"]
                        if all_add:
                            alo = min(jj * QC + w0
                                      for j, jj, w0, w1, live in out)
                            nc.vector.tensor_add(
                                pacc[:, alo:hi], pacc[:, alo:hi],
                                P[:, alo:hi])
                        else:
                            for j, jj, w0, w1, live in out:
                                if ctx["first_i"][j] == i:
                                    if i == pair_first and jj * QC > 0 \
                                            and j == out[0][0]:
                                        nc.vector.memset(
                                            pacc[:, 0:jj * QC], 0.0)
                                    nc.vector.tensor_copy(
                                        pacc[:, jj * QC:(jj + 1) * QC],
                                        P[:, jj * QC:(jj + 1) * QC])
                                else:
                                    nc.vector.tensor_add(
                                        pacc[:, w0 + jj * QC:
                                             (jj + 1) * QC],
                                        pacc[:, w0 + jj * QC:
                                             (jj + 1) * QC],
                                        P[:, w0 + jj * QC:(jj + 1) * QC])
                        return res

                    prev = None
                    for i in sorted(blocks_j):
                        cur = (ctx, i, emit_s(i, blocks_j[i]))
                        tick_norms()
                        flush_pending()
                        if prev is not None:
                            emit_ovr(*prev)
                        prev = cur
                        if jpair[0] == 2:
                            it23 += 1
                            if it23 >= 6 and (it23 - 6) % 7 == 0 \
                                    and op_units:
                                op_units.pop(0)()
                    if prev is not None:
                        pending.append(prev)
            flush_pending(final=True)
            # leftover chunk-0/1 units, then the chunk-2/3 projection
            for u in op_units:
                u()
            for j in (2, 3):
                for op in range(H // 256):
                    op_unit(j, op)()


def _prep_inputs(x, mask, Wq, Wk, Wv, Wo, patterns):
    import ml_dtypes
    bf16 = ml_dtypes.bfloat16

    # RoPE tables, d-major [HD, L]
    inv_freq = 1.0 / (ROPE_BASE ** (np.arange(0, HD, 2, dtype=np.float64)
                                    / HD))
    t = np.arange(L, dtype=np.float64)
    freqs = np.outer(t, inv_freq)                     # [L, HD/2]
    emb = np.concatenate((freqs, freqs), axis=-1)     # [L, HD]
    cos = np.cos(emb).T.astype(np.float32)            # [HD, L]
    sin = np.sin(emb).T.astype(np.float32)
    sinm = sin.copy()
    sinm[0:64] = -sin[0:64]
    cos_b = cos.astype(bf16)
    sinm_b = sinm.astype(bf16)

    npat = max(len(patterns), 1)
    maskd = np.zeros((npat, 128, 128), dtype=bf16)
    for i, p in enumerate(patterns):
        maskd[i] = p.astype(np.float32).astype(bf16)

    def wprep(wT):
        # [K, M] (contraction-major) -> [128, K//128, M] partition-major
        # (fat DMA packets)
        wT = np.ascontiguousarray(wT).astype(bf16)
        k, m = wT.shape
        return np.ascontiguousarray(
            wT.reshape(k // 128, 128, m).transpose(1, 0, 2))

    NQCl = L // 512
    in_maps = []
    for c in range(NCORES):
        b, half = c // 2, c % 2
        rows = slice(half * HPC * HD, (half + 1) * HPC * HD)
        xT = np.ascontiguousarray(x[b].T).astype(bf16)    # [H, L]
        # [NQC, 128, (H//128)*512]: per chunk, per partition, all 16
        # feature blocks contiguous
        xP = np.ascontiguousarray(
            xT.reshape(H // 128, 128, NQCl, 512)
            .transpose(2, 1, 0, 3)
            .reshape(NQCl, 128, (H // 128) * 512))
        in_maps.append({
            "xP": xP,
            "wqT": wprep(Wq[rows, :].T),
            "wkT": wprep(Wk[rows, :].T),
            "wvT": wprep(Wv[rows, :].T),
            "woT": wprep(Wo[:, rows].T),
            "cosd": cos_b,
            "sinmd": sinm_b,
            "maskd": maskd,
        })
    return in_maps


def kernel(x, mask, Wq, Wk, Wv, Wo, _trace=False):
    from concourse.bass_utils import run_bass_kernel_spmd

    x = np.asarray(x, dtype=np.float32)
    mask2d = np.asarray(mask, dtype=np.int32).reshape(L, L)
    key = mask2d.tobytes()
    if key not in _cache:
        kind, patterns, block_pat = _analyze_mask(mask2d)
        nc = _build(kind, block_pat, len(patterns))
        _cache[key] = (nc, patterns)
    nc, patterns = _cache[key]

    in_maps = _prep_inputs(x, mask, np.asarray(Wq, np.float32),
                           np.asarray(Wk, np.float32),
                           np.asarray(Wv, np.float32),
                           np.asarray(Wo, np.float32), patterns)
    res = run_bass_kernel_spmd(nc, in_maps, list(range(NCORES)),
                               trace=_trace)
    y = np.empty((B, L, H), dtype=np.float32)
    for b in range(B):
        acc = res.results[2 * b]["yT"].astype(np.float32) + \
              res.results[2 * b + 1]["yT"].astype(np.float32)
        y[b] = acc.T
    if _trace:
        kernel.last_results = res
    return y


if __name__ == "__main__":
    import reference
    inputs = reference.setup_inputs()
    inputs = {k: np.asarray(v) for k, v in inputs.items()}
    out = kernel(**inputs)
    exp = np.asarray(reference.reference(**{k: v for k, v in inputs.items()}))
    err = np.abs(out - exp).max() / np.abs(exp).max()
    print("rel err (absmax):", err)



# revision 51
# speedup vs baseline: 1.0806x; 1.0002x over previous
"""Trainium2 Bass kernel: causal multi-head attention with RoPE.

Model: B=4, L=2048, H=2048, NH=16 heads, head_dim=128.
  q = x @ Wq.T ; k = x @ Wk.T ; v = x @ Wv.T        (per-head split)
  q, k <- RoPE(q, k)
  attn = softmax(mask(q k^T / sqrt(hd)))
  out  = (attn @ v) heads-concat @ Wo.T

Sharding (8 cores): hybrid batch x tensor-parallel.  Core c handles
batch b = c//2 and heads half*8..half*8+7 with half = c%2.  Wq/Wk/Wv are
column-sharded (8 heads per core), Wo row-sharded; each core produces a
partial y[b] and the host sums the two partials per batch (the unshard
step) and concatenates batches.

Per-core dataflow (all SBUF-resident, bf16 inputs / fp32 accumulation):
  phase A: Q^T, K^T  [128d x 2048pos] per head (d-major) and V
           [128pos x 1024d] pos-major, via PE matmuls; RoPE on Q^T/K^T
           (rotate-half partition shuffle via SBUF->SBUF DMA, the
           elementwise part on DVE).
  phase B: flash-style causal attention per (head, 512-wide q chunk):
           S^T tile = K_blk^T Q_chunk (PE), P = exp(S^T/sqrt(d)) (ACT),
           block-sparse causal structure with a triangular-mask multiply
           on diagonal blocks (DVE), O^T += V_blk P (PE), rowsum via
           ones-matmul (PE), reciprocal+broadcast+scale for the softmax
           normalization (DVE + GPSIMD).
  phase C: y^T partial = Wo_shard O^T (PE) -> DRAM fp32.
"""

import math
import numpy as np

B, L, H, NH, HD = 4, 2048, 2048, 16, 128
ROPE_BASE = 10000.0
NCORES = 8
HPC = 8          # heads per core
QC = 512         # q chunk width
NQC = L // QC    # 4 q chunks
NKB = L // 128   # 16 kp blocks
SCALE = 1.0 / math.sqrt(HD)

_cache = {}


def _analyze_mask(mask2d):
    """Classify each (q_block, kp_block) 128x128 block of the [L, L] mask.

    Returns (block_kind[16][16] with 0=empty,1=full,2=mixed, patterns,
    pattern_idx dict keyed by block coords). mask2d is int32 [L, L],
    rows=q, cols=kp.
    """
    nb = L // 128
    kind = [[0] * nb for _ in range(nb)]
    patterns = []
    pat_key_to_idx = {}
    block_pat = {}
    for qb in range(nb):
        rows = mask2d[qb * 128:(qb + 1) * 128]
        for kb in range(nb):
            blk = rows[:, kb * 128:(kb + 1) * 128]
            s = int(blk.sum())
            if s == 0:
                kind[qb][kb] = 0
            elif s == 128 * 128:
                kind[qb][kb] = 1
            else:
                kind[qb][kb] = 2
                key = blk.tobytes()
                idx = pat_key_to_idx.get(key)
                if idx is None:
                    idx = len(patterns)
                    pat_key_to_idx[key] = idx
                    # stored transposed: S^T tiles are [kp, q]
                    patterns.append(np.ascontiguousarray(blk.T))
                block_pat[(qb, kb)] = idx
    return kind, patterns, block_pat


def _build(kind, block_pat, n_patterns):
    """Build the SPMD bass program (same for all 8 cores)."""
    import concourse.bass as bass
    import concourse.bacc as bacc
    import concourse.mybir as mybir
    import concourse.tile as tile

    fp32 = mybir.dt.float32
    bf16 = mybir.dt.bfloat16
    EXP = mybir.ActivationFunctionType.Exp

    nc = bacc.Bacc("TRN2", target_bir_lowering=False, debug=False)

    NHC = H // 128  # 16 input-feature blocks

    # All big operands are pre-shuffled on the host into partition-major
    # layouts with LONG contiguous per-partition runs so DMA packets are
    # 8-32KB instead of 1-2KB (per-queue DMA throughput is packet-rate
    # limited).
    xP = nc.dram_tensor("xP", [NQC, 128, NHC * QC], bf16,
                        kind="ExternalInput")
    wqT = nc.dram_tensor("wqT", [128, NHC, HPC * HD], bf16,
                         kind="ExternalInput")
    wkT = nc.dram_tensor("wkT", [128, NHC, HPC * HD], bf16,
                         kind="ExternalInput")
    wvT = nc.dram_tensor("wvT", [128, NHC, HPC * HD], bf16,
                         kind="ExternalInput")
    woT = nc.dram_tensor("woT", [128, HPC, H], bf16, kind="ExternalInput")
    cosd = nc.dram_tensor("cosd", [HD, L], bf16, kind="ExternalInput")
    sinmd = nc.dram_tensor("sinmd", [HD, L], bf16, kind="ExternalInput")
    npat = max(n_patterns, 1)
    maskd = nc.dram_tensor("maskd", [npat, 128, 128], bf16, kind="ExternalInput")
    yT = nc.dram_tensor("yT", [H, L], bf16, kind="ExternalOutput")

    def qk_phase(tc, w_dram, out_a, wpool, xpool, tpool, pspool, wtag,
                 cos_sb, sinm_sb, delay_after=None, after_x0=None):
        """Q^T / K^T d-major projection + fused RoPE per (head, chunk).

        delay_after: instructions the weight DMA must wait for -- used to
        keep the K-phase weight prefetch off the critical head-of-kernel
        DMA bandwidth.  Weight DMAs ride the scalar HWDGE queue so they
        run concurrently with x DMAs on the sync queue.
        """
        from concourse.tile import add_dep_helper
        w_sb = wpool.tile([128, NHC, HPC * HD], bf16, tag="w",
                          name=f"w_{wtag}")
        w_insts = []
        first = wtag == "q"
        # head-of-kernel: interleave the first weight/x groups across the
        # two HWDGE queues in consumption order so hc-group g's operands
        # land just in time (w g1/g3 are emitted after x j0 below)
        for g in ((0, 2) if first else (0, 1, 2, 3)):
            w_insts.append(
                nc.scalar.dma_start(out=w_sb[:, 4 * g:4 * g + 4, :],
                                    in_=w_dram[:, 4 * g:4 * g + 4, :]))
        if delay_after:
            for wi in w_insts:
                for di in delay_after:
                    add_dep_helper(wi.ins, di.ins, reason="defer weight prefetch")
        x0_insts = []
        # RoPE for head h is emitted one head late so the DVE queue (strict
        # 8-deep FIFO) never head-blocks on the rotate-half DMA latency.
        rope_q = []

        def emit_rope(h, js):
            q = out_a[:, h, js]
            rq = tpool.tile([128, QC], bf16, tag="rotq")
            eng = nc.sync if h % 2 == 0 else nc.scalar
            eng.dma_start(out=rq[0:64, :], in_=out_a[64:128, h, js])
            eng.dma_start(out=rq[64:128, :], in_=out_a[0:64, h, js])

            def fire():
                nc.vector.tensor_mul(rq[:], rq[:], sinm_sb[:, js])
                nc.vector.tensor_mul(q, q, cos_sb[:, js])
                nc.vector.tensor_add(q, q, rq[:])
            rope_q.append(fire)
            if len(rope_q) > 1:
                rope_q.pop(0)()

        for j in range(NQC):
            js = slice(j * QC, (j + 1) * QC)
            x_sb = xpool.tile([128, NHC, QC], bf16, tag="xcols",
                              name=f"x_{wtag}{j}")
            if first and j == 0:
                for g in (0, 2):
                    x0_insts.append(nc.sync.dma_start(
                        out=x_sb[:, 4 * g:4 * g + 4, :],
                        in_=xP[j, :, 4 * g * QC:(4 * g + 4) * QC]))
                for g in (1, 3):
                    x0_insts.append(nc.scalar.dma_start(
                        out=x_sb[:, 4 * g:4 * g + 4, :],
                        in_=xP[j, :, 4 * g * QC:(4 * g + 4) * QC]))
                for g in (1, 3):
                    nc.sync.dma_start(
                        out=w_sb[:, 4 * g:4 * g + 4, :],
                        in_=w_dram[:, 4 * g:4 * g + 4, :])
            else:
                for g in range(4):
                    di = nc.sync.dma_start(
                        out=x_sb[:, 4 * g:4 * g + 4, :],
                        in_=xP[j, :, 4 * g * QC:(4 * g + 4) * QC])
                    if j == 0:
                        x0_insts.append(di)
            if j == 0 and after_x0 is not None:
                after_x0()
            for h in range(HPC):
                ps = pspool.tile([128, QC], fp32, tag="ps_proj")
                for hc in range(NHC):
                    nc.tensor.matmul(
                        ps[:],
                        w_sb[:, hc, h * HD:(h + 1) * HD],
                        x_sb[:, hc, :],
                        start=(hc == 0), stop=(hc == NHC - 1))
                nc.vector.tensor_copy(out_a[:, h, js], ps[:])
                emit_rope(h, js)
        while rope_q:
            rope_q.pop(0)()
        return x0_insts

    def v_phase(tc, w_dram, va, wpool, xpool, pspool, delay_after=None):
        """V pos-major projection (same pre-shuffled x chunks as Q/K)."""
        from concourse.tile import add_dep_helper
        w_sb = wpool.tile([128, NHC, HPC * HD], bf16, tag="w", name="w_v")
        for g in range(4):
            nc.scalar.dma_start(out=w_sb[:, 4 * g:4 * g + 4, :],
                                in_=w_dram[:, 4 * g:4 * g + 4, :])
        for j in range(NQC):
            x_sb = xpool.tile([128, NHC, QC], bf16, tag="xv", name=f"xv{j}")
            for g in range(4):
                di = nc.sync.dma_start(
                    out=x_sb[:, 4 * g:4 * g + 4, :],
                    in_=xP[j, :, 4 * g * QC:(4 * g + 4) * QC])
                if delay_after and j == 0:
                    for d0 in delay_after:
                        add_dep_helper(di.ins, d0.ins,
                                       reason="defer xv prefetch")
            for pb in range(QC // 128):
                psd = [pspool.tile([128, QC], fp32, tag="ps_proj",
                                   name=f"psv{j}_{pb}_{dc}")
                       for dc in range(2)]
                for hc in range(NHC):
                    for dc in range(2):
                        nc.tensor.matmul(
                            psd[dc][:],
                            x_sb[:, hc, pb * 128:(pb + 1) * 128],
                            w_sb[:, hc, dc * QC:(dc + 1) * QC],
                            start=(hc == 0), stop=(hc == NHC - 1))
                for dc in range(2):
                    nc.vector.tensor_copy(
                        va[:, j * (QC // 128) + pb, dc * QC:(dc + 1) * QC],
                        psd[dc][:])

    with tile.TileContext(nc) as tc:
        with tc.tile_pool(name="persist", bufs=1, side="left") as persist:
            # one combined small-constant tile: [trimask patterns | ones]
            cst = persist.tile([128, npat * 128 + 128], bf16, tag="cst")
            ones_col = npat * 128
            nc.vector.memset(cst[:, ones_col:ones_col + 128], 1.0)
            onesf = persist.tile([128, 128], fp32, tag="onesf")
            nc.vector.memset(onesf[:], 1.0)
            # warm the gpsimd partition-ops library now (PE is idle waiting
            # on DMAs anyway); the first real partition_broadcast otherwise
            # stalls attention ~9us on LOAD_LIB.
            nc.gpsimd.partition_broadcast(onesf[:, 0:8], onesf[0:1, 0:8])
            QTa = persist.tile([HD, HPC, L], bf16, tag="qta")
            KTa = persist.tile([HD, HPC, L], bf16, tag="kta")

            # ---------------- phase A: projections + RoPE ----------------
            # Manual pool lifetimes (non-LIFO): weights/x/rope tables are
            # freed before attention while Va spans V-phase..attention.
            wpool_cm = tc.tile_pool(name="wpool", bufs=2, side="right")
            wpool = wpool_cm.__enter__()
            ropec_cm = tc.tile_pool(name="ropec", bufs=1, side="right")
            ropec = ropec_cm.__enter__()
            psp_cm = tc.tile_pool(name="ps_proj", bufs=4, space="PSUM")
            psp = psp_cm.__enter__()

            cos_sb = ropec.tile([HD, L], bf16, tag="cos")
            sinm_sb = ropec.tile([HD, L], bf16, tag="sinm")

            def rope_dma():
                # emitted after the first x chunk so the critical-path
                # startup DMAs (x g0 + wq g0) own the HBM bandwidth
                nc.sync.dma_start(out=cos_sb[:], in_=cosd[:])
                nc.sync.dma_start(out=sinm_sb[:], in_=sinmd[:])

            xv_cm = tc.tile_pool(name="xv", bufs=2, side="right")
            xv = xv_cm.__enter__()
            xqk_cm = tc.tile_pool(name="xqk", bufs=2, side="right")
            xqk = xqk_cm.__enter__()
            tpool_cm = tc.tile_pool(name="tpool", bufs=2, side="right")
            tpool = tpool_cm.__enter__()
            q_x0 = qk_phase(tc, wqT, QTa, wpool, xqk, tpool, psp, "q",
                            cos_sb, sinm_sb, after_x0=rope_dma)
            qk_phase(tc, wkT, KTa, wpool, xqk, tpool, psp, "k",
                     cos_sb, sinm_sb, delay_after=q_x0)
            # tri-mask patterns: needed only in attention; scalar queue
            # behind the wq/wk weight loads
            for p in range(n_patterns):
                nc.scalar.dma_start(out=cst[:, p * 128:(p + 1) * 128],
                                    in_=maskd[p])
            tpool_cm.__exit__(None, None, None)
            xqk_cm.__exit__(None, None, None)
            vp_cm = tc.tile_pool(name="vp", bufs=1, side="left")
            vp_outer = vp_cm.__enter__()
            Va = vp_outer.tile([128, NKB, HPC * HD], bf16, tag="va")
            v_phase(tc, wvT, Va, wpool, xv, psp, delay_after=q_x0)
            xv_cm.__exit__(None, None, None)
            ropec_cm.__exit__(None, None, None)
            wpool_cm.__exit__(None, None, None)
            psp_cm.__exit__(None, None, None)

            # -------- phase B + C under Va's lifetime --------
            _attn_and_out(tc, nc, kind, block_pat, QTa, KTa, Va,
                          cst, ones_col, onesf, woT, yT, fp32, bf16, EXP)
            vp_cm.__exit__(None, None, None)

    nc.compile()
    return nc


def _attn_and_out(tc, nc, kind, block_pat, QTa, KTa, Va, cst, ones_col,
                  onesf, woT, yT, fp32, bf16, EXP):
    ones_sb = cst[:, ones_col:ones_col + 1]
    with tc.tile_pool(name="otp", bufs=1, side="left") as otp, \
         tc.tile_pool(name="wo", bufs=1, side="left") as wop:
        OTa = otp.tile([HD, HPC, L], bf16, tag="ota")
        wo_sb = wop.tile([128, HPC, H], bf16, tag="wo")
        # prefetch Wo during attention, split across both HWDGE queues
        nc.scalar.dma_start(out=wo_sb[:, 0:4, :], in_=woT[:, 0:4, :])
        nc.sync.dma_start(out=wo_sb[:, 4:8, :], in_=woT[:, 4:8, :])

        # ---------------- phase B: attention ----------------
        # q-chunk PAIRS inside the kp-block loop.  Per (i, jpair) both S
        # tiles land in one 2-bank PSUM tile so a single ACT exp covers
        # them (ACT cost is (N+~310)/1.2 ns -- instruction count matters);
        # the O accumulators and the running softmax-denominator Pacc are
        # pair-wide too.  The denominator is finished by a gpsimd
        # partition_all_reduce (cross-partition sum, idle engine) followed
        # by a DVE reciprocal, and the normalization multiplies PSUM O by
        # the all-reduced reciprocal directly -- no ones-matmul, no
        # partition_broadcast, no PSUM rowsum banks.  The last O/normalize
        # group of each pair is deferred into the next pair's instruction
        # stream (software pipeline) so the PE never head-of-line blocks.
        import concourse.bass_isa as bass_isa
        with tc.tile_pool(name="pp", bufs=4, side="right") as ppool, \
             tc.tile_pool(name="aa", bufs=2, side="right") as apool, \
             tc.tile_pool(name="rs", bufs=2, side="right") as rspool, \
             tc.tile_pool(name="ysb", bufs=6, side="right") as ypool, \
             tc.tile_pool(name="ps_s", bufs=2, space="PSUM") as ps_s, \
             tc.tile_pool(name="ps_o", bufs=1, space="PSUM") as ps_o, \
             tc.tile_pool(name="ps_r", bufs=2, space="PSUM") as ps_r:
            pending = []   # deferred last-O emissions (flushed next iter)
            norm_q = []    # deferred recip+normalize closures, with age

            # ---- output-projection units (phase C, interleavable) ----
            # one unit = (q-chunk j, pair of output blocks): 16 matmuls
            # into a 2-bank PSUM tile from the shared pss ring, one ACT
            # copy to SBUF, one fat DMA out.  Chunk 0/1 units interleave
            # into the (2,3)-pair attention stream; chunk 2/3 units run
            # after attention.
            def op_unit(j, op):
                def emit():
                    ps = ps_s.tile([128, 2 * QC], fp32, tag="pss",
                                   name=f"psc{j}_{op}")
                    for koc in range(2):
                        oc = 2 * op + koc
                        for fc in range(HPC):
                            nc.tensor.matmul(
                                ps[:, koc * QC:(koc + 1) * QC],
                                wo_sb[:, fc, oc * 128:(oc + 1) * 128],
                                OTa[:, fc, j * QC:(j + 1) * QC],
                                start=(fc == 0), stop=(fc == HPC - 1))
                    y_sb = ypool.tile([128, 2 * QC], bf16, tag="y")
                    nc.scalar.copy(y_sb[:], ps[:])
                    yr = yT[2 * op * 128:(2 * op + 2) * 128,
                            j * QC:(j + 1) * QC].rearrange(
                                "(a p) m -> p a m", p=128)
                    eng = nc.sync if op % 2 == 0 else nc.scalar
                    eng.dma_start(out=yr, in_=y_sb[:])
                return emit

            op_units = [op_unit(j, op) for j in (0, 1)
                        for op in range(H // 256)]

            def emit_ovr(ctx, i, group):
                h = ctx["h"]
                # O matmuls first (V stationary shared across the pair)
                for j, jj, P, w0, first in group:
                    m0 = 0 if first else w0
                    nc.tensor.matmul(
                        ctx["pso"][:, jj * QC + m0:(jj + 1) * QC],
                        Va[:, i, h * HD:(h + 1) * HD],
                        P[:, jj * QC + m0:(jj + 1) * QC],
                        start=first, stop=(ctx["last_i"][j] == i))
                if i == ctx["pair_last"]:
                    # evacuate O to SBUF on ACT right away: frees the pso
                    # PSUM tile so the next pairs never wait on the
                    # normalization chain
                    oev = rspool.tile([128, 2 * QC], bf16, tag="oev",
                                      name=f"oev{h}_{ctx['jp0']}")
                    nc.scalar.copy(oev[:], ctx["pso"][:])
                    # denominators: two cheap ones-matmuls over the
                    # accumulated Pacc halves (contraction over kp)
                    psr = {}
                    for jj in range(2):
                        psr[jj] = ps_r.tile([1, QC], fp32, tag="psr",
                                            name=f"psr{h}_{ctx['jp0']}{jj}")
                        nc.tensor.matmul(
                            psr[jj][0:1, :], ones_sb,
                            ctx["pacc"][:, jj * QC:(jj + 1) * QC],
                            start=True, stop=True)

                    def norm(ctx=ctx, psr=psr, oev=oev, h=h):
                        rp = rspool.tile([1, 2 * QC], fp32, tag="rp",
                                         name=f"rp{h}_{ctx['jp0']}")
                        for jj in range(2):
                            nc.vector.reciprocal_approx_fast(
                                out=rp[0:1, jj * QC:(jj + 1) * QC],
                                in_=psr[jj][0:1, :])
                        rb = rspool.tile([1, 2 * QC], bf16, tag="rb",
                                         name=f"rb{h}_{ctx['jp0']}")
                        nc.vector.tensor_copy(rb[0:1, :], rp[0:1, :])
                        bc = rspool.tile([128, 2 * QC], bf16, tag="bc",
                                         name=f"bc{h}_{ctx['jp0']}")
                        nc.gpsimd.partition_broadcast(bc[:], rb[0:1, :])
                        nc.vector.tensor_mul(
                            OTa[:, h,
                                ctx["jp0"] * QC:(ctx["jp0"] + 2) * QC],
                            oev[:], bc[:])
                    norm_q.append([0, norm])

            def tick_norms(final=False):
                for e in norm_q:
                    e[0] += 1
                while norm_q and (final or norm_q[0][0] >= 3):
                    norm_q.pop(0)[1]()

            def flush_pending(final=False):
                while pending:
                    emit_ovr(*pending.pop(0))
                if final:
                    tick_norms(final=True)

            it23 = 0
            for jpair, h in [((0, 1), hh) for hh in range(HPC)] + \
                            [((2, 3), hh) for hh in range(HPC)]:
                if True:
                    blocks_j = {}
                    first_i = {}
                    last_i = {}
                    for j in jpair:
                        for i in range(NKB):
                            live = [t for t in range(4)
                                    if kind[4 * j + t][i] != 0]
                            if live:
                                blocks_j.setdefault(i, []).append((j, live))
                                if j not in first_i:
                                    first_i[j] = i
                                last_i[j] = i
                    if not first_i:
                        continue
                    pair_first = min(first_i.values())
                    ctx = {
                        "h": h,
                        "jp0": jpair[0],
                        "first_i": first_i,
                        "last_i": last_i,
                        "pair_last": max(last_i.values()),
                        "pso": ps_o.tile([128, 2 * QC], fp32, tag="pso",
                                         name=f"pso{h}_{jpair[0]}"),
                        "pacc": apool.tile([128, 2 * QC], bf16, tag="pacc",
                                           name=f"pacc{h}_{jpair[0]}"),
                    }

                    def emit_s(i, group):
                        # one [128, 2*QC] PSUM tile for the pair's S tiles
                        pss = ps_s.tile([128, 2 * QC], fp32, tag="pss",
                                        name=f"pss{ctx['h']}_{i}")
                        out = []
                        lo, hi = None, None
                        for j, live in group:
                            jj = j - jpair[0]
                            t0, t1 = live[0], live[-1]
                            w0, w1 = t0 * 128, (t1 + 1) * 128
                            nc.tensor.matmul(
                                pss[:, jj * QC + w0:jj * QC + w1],
                                KTa[:, ctx["h"], i * 128:(i + 1) * 128],
                                QTa[:, ctx["h"],
                                    j * QC + w0:j * QC + w1],
                                start=True, stop=True)
                            if lo is None:
                                lo = jj * QC + w0
                            hi = jj * QC + w1
                            out.append((j, jj, w0, w1, live))
                        P = ppool.tile([128, 2 * QC], bf16, tag="p",
                                       name=f"p{ctx['h']}_{i}")
                        # single exp over the pair's contiguous live span
                        nc.scalar.activation(P[:, lo:hi], pss[:, lo:hi],
                                             EXP, scale=SCALE)
                        res = []
                        # pair-wide add only when live spans are contiguous
                        all_add = all(
                            out[k][1] * QC + out[k][3] ==
                            out[k + 1][1] * QC + out[k + 1][2]
                            for k in range(len(out) - 1))
                        for j, jj, w0, w1, live in out:
                            first = (ctx["first_i"][j] == i)
                            if first:
                                all_add = False
                            if w0 > 0 and first:
                                nc.vector.memset(P[:, jj * QC:jj * QC + w0],
                                                 0.0)
                            if w1 < QC and first:
                                nc.vector.memset(
                                    P[:, jj * QC + w1:(jj + 1) * QC], 0.0)
                            for t in range(live[0], live[-1] + 1):
                                qb = 4 * j + t
                                base = jj * QC + t * 128
                                if kind[qb][i] == 0:
                                    nc.vector.memset(
                                        P[:, base:base + 128], 0.0)
                                elif kind[qb][i] == 2:
                                    pat = block_pat[(qb, i)]
                                    nc.vector.tensor_mul(
                                        P[:, base:base + 128],
                                        P[:, base:base + 128],
                                        cst[:, pat * 128:(pat + 1) * 128])
                            res.append((j, jj, P, w0, first))
                        # running softmax-denominator accumulation (DVE),
                        # one pair-wide op when possible
                        pacc = ctx["pacc"]
                        if all_add:
                            alo = min(jj * QC + w0
                                      for j, jj, w0, w1, live in out)
                            nc.vector.tensor_add(
                                pacc[:, alo:hi], pacc[:, alo:hi],
                                P[:, alo:hi])
                        else:
                            for j, jj, w0, w1, live in out:
                                if ctx["first_i"][j] == i:
                                    if i == pair_first and jj * QC > 0 \
                                            and j == out[0][0]:
                                        nc.vector.memset(
                                            pacc[:, 0:jj * QC], 0.0)
                                    nc.vector.tensor_copy(
                                        pacc[:, jj * QC:(jj + 1) * QC],
                                        P[:, jj * QC:(jj + 1) * QC])
                                else:
                                    nc.vector.tensor_add(
                                        pacc[:, w0 + jj * QC:
                                             (jj + 1) * QC],
                                        pacc[:, w0 + jj * QC:
                                             (jj + 1) * QC],
                                        P[:, w0 + jj * QC:(jj + 1) * QC])
                        return res

                    prev = None
                    for i in sorted(blocks_j):
                        cur = (ctx, i, emit_s(i, blocks_j[i]))
                        tick_norms()
                        flush_pending()
                        if prev is not None:
                            emit_ovr(*prev)
                        prev = cur
                        if jpair[0] == 2:
                            it23 += 1
                            if it23 >= 6 and (it23 - 6) % 7 == 0 \
                                    and op_units:
                                op_units.pop(0)()
                    if prev is not None:
                        pending.append(prev)
            while pending:
                emit_ovr(*pending.pop(0))
            # epilogue: interleave the remaining normalizations with the
            # leftover chunk-0/1 units and the chunk-2/3 projection so the
            # ACT/DVE backlog never bursts in front of the PE stream
            tail_units = op_units + [op_unit(j, op) for j in (2, 3)
                                     for op in range(H // 256)]
            for u in tail_units:
                if norm_q:
                    norm_q.pop(0)[1]()
                u()
            tick_norms(final=True)


def _prep_inputs(x, mask, Wq, Wk, Wv, Wo, patterns):
    import ml_dtypes
    bf16 = ml_dtypes.bfloat16

    # RoPE tables, d-major [HD, L]
    inv_freq = 1.0 / (ROPE_BASE ** (np.arange(0, HD, 2, dtype=np.float64)
                                    / HD))
    t = np.arange(L, dtype=np.float64)
    freqs = np.outer(t, inv_freq)                     # [L, HD/2]
    emb = np.concatenate((freqs, freqs), axis=-1)     # [L, HD]
    cos = np.cos(emb).T.astype(np.float32)            # [HD, L]
    sin = np.sin(emb).T.astype(np.float32)
    sinm = sin.copy()
    sinm[0:64] = -sin[0:64]
    cos_b = cos.astype(bf16)
    sinm_b = sinm.astype(bf16)

    npat = max(len(patterns), 1)
    maskd = np.zeros((npat, 128, 128), dtype=bf16)
    for i, p in enumerate(patterns):
        maskd[i] = p.astype(np.float32).astype(bf16)

    def wprep(wT):
        # [K, M] (contraction-major) -> [128, K//128, M] partition-major
        # (fat DMA packets)
        wT = np.ascontiguousarray(wT).astype(bf16)
        k, m = wT.shape
        return np.ascontiguousarray(
            wT.reshape(k // 128, 128, m).transpose(1, 0, 2))

    NQCl = L // 512
    in_maps = []
    for c in range(NCORES):
        b, half = c // 2, c % 2
        rows = slice(half * HPC * HD, (half + 1) * HPC * HD)
        xT = np.ascontiguousarray(x[b].T).astype(bf16)    # [H, L]
        # [NQC, 128, (H//128)*512]: per chunk, per partition, all 16
        # feature blocks contiguous
        xP = np.ascontiguousarray(
            xT.reshape(H // 128, 128, NQCl, 512)
            .transpose(2, 1, 0, 3)
            .reshape(NQCl, 128, (H // 128) * 512))
        in_maps.append({
            "xP": xP,
            "wqT": wprep(Wq[rows, :].T),
            "wkT": wprep(Wk[rows, :].T),
            "wvT": wprep(Wv[rows, :].T),
            "woT": wprep(Wo[:, rows].T),
            "cosd": cos_b,
            "sinmd": sinm_b,
            "maskd": maskd,
        })
    return in_maps


def kernel(x, mask, Wq, Wk, Wv, Wo, _trace=False):
    from concourse.bass_utils import run_bass_kernel_spmd

    x = np.asarray(x, dtype=np.float32)
    mask2d = np.asarray(mask, dtype=np.int32).reshape(L, L)
    key = mask2d.tobytes()
    if key not in _cache:
        kind, patterns, block_pat = _analyze_mask(mask2d)
        nc = _build(kind, block_pat, len(patterns))
        _cache[key] = (nc, patterns)
    nc, patterns = _cache[key]

    in_maps = _prep_inputs(x, mask, np.asarray(Wq, np.float32),
                           np.asarray(Wk, np.float32),
                           np.asarray(Wv, np.float32),
                           np.asarray(Wo, np.float32), patterns)
    res = run_bass_kernel_spmd(nc, in_maps, list(range(NCORES)),
                               trace=_trace)
    y = np.empty((B, L, H), dtype=np.float32)
    for b in range(B):
        acc = res.results[2 * b]["yT"].astype(np.float32) + \
              res.results[2 * b + 1]["yT"].astype(np.float32)
        y[b] = acc.T
    if _trace:
        kernel.last_results = res
    return y


if __name__ == "__main__":
    import reference
    inputs = reference.setup_inputs()
    inputs = {k: np.asarray(v) for k, v in inputs.items()}
    out = kernel(**inputs)
    exp = np.asarray(reference.reference(**{k: v for k, v in inputs.items()}))
    err = np.abs(out - exp).max() / np.abs(exp).max()
    print("rel err (absmax):", err)

